# revision 24
# baseline (speedup 1.0000x reference)
"""GQA attention (32 Q heads / 8 KV heads, head_dim 128, d_model 2560, s=2048)
with RoPE, tensor-parallel across 8 TRN2 NeuronCores.

Sharding: core i owns Q heads 4i..4i+3 and KV head i (w_q/w_k/w_v sharded on
the head output dim, w_o on its input dim). Each core produces a full-shape
partial of the output projection; the partials are summed on the host (the
"all-reduce after w_o" of the hint, done at unshard time), so no on-device
collective is needed and outputs (k/v caches, out rows) are disjoint/partial.

Precision: the softmax logits here have std ~2.5e3 (softmax is near one-hot),
so Q/K projections and QK^T are computed as 3-pass split-bf16 matmuls
(a1b1+a1b2+a2b1, fp32-equivalent to ~2^-18) while the V chain (V proj,
attn@V, O proj) runs in plain bf16. Measured end-to-end rel err ~3e-3.

Layout: everything runs transposed ([dim, seq] on chip) so that projections,
scores^T, attn@V and the O projection all chain without any transposes.
Softmax runs over the partition axis via gpsimd C-axis reduce + broadcast.
"""
import sys
sys.path.insert(0, '/opt/trn_rl_repo')

import numpy as np
import ml_dtypes

import concourse.bass as bass
import concourse.tile as tile
from concourse import bacc, mybir
from concourse.bass_utils import run_bass_kernel_spmd

BF16 = ml_dtypes.bfloat16
F32 = mybir.dt.float32
BF = mybir.dt.bfloat16

D_MODEL = 2560
NUM_HEADS = 32
NUM_KV = 8
HD = 128
S = 2048
THETA = 5000000.0
NC_CORES = 8
NDT = D_MODEL // 128      # 20 contraction tiles
NSB = S // 512            # 4 sequence blocks of 512
NTT = S // 128            # 16 key tiles of 128
NM = 4                    # q heads per core
SCALE = 1.0 / np.sqrt(HD)
NEG = -1.0e30

_cache = {}


def _build():
    nc = bacc.Bacc("TRN2", target_bir_lowering=False, debug=False,
                   num_devices=NC_CORES)
    dp = nc.declare_dram_parameter
    x1_e = dp("x1", [NSB, NDT, 128, 512], BF, isOutput=False)
    x2_e = dp("x2", [NSB, NDT, 128, 512], BF, isOutput=False)
    wq1_e = dp("wq1", [NM, 128, NDT * 128], BF, isOutput=False)
    wq2_e = dp("wq2", [NM, 128, NDT * 128], BF, isOutput=False)
    wk1_e = dp("wk1", [128, NDT * 128], BF, isOutput=False)
    wk2_e = dp("wk2", [128, NDT * 128], BF, isOutput=False)
    wv_e = dp("wv", [128, NDT * 128], BF, isOutput=False)
    wo_e = dp("wo", [128, NM * NDT * 128], BF, isOutput=False)
    cos_e = dp("cosT", [128, S], F32, isOutput=False)  # rows 0:64 cos, 64:128 sin
    msk_e = dp("masks", [128, 4 * 512], BF, isOutput=False)
    out_e = dp("outT", [NSB, NDT, 128, 512], F32, isOutput=True)
    kt_e = dp("kT", [128, S], F32, isOutput=True)
    v_e = dp("vout", [NTT, 128, 128], BF, isOutput=True)

    with tile.TileContext(nc) as tc:
        _body(nc, tc, x1_e, x2_e, wq1_e, wq2_e, wk1_e, wk2_e, wv_e, wo_e,
              cos_e, msk_e, out_e, kt_e, v_e)
    nc.compile()
    return nc


def _body(nc, tc, x1_e, x2_e, wq1_e, wq2_e, wk1_e, wk2_e, wv_e, wo_e,
          cos_e, msk_e, out_e, kt_e, v_e):
    AF = mybir.ActivationFunctionType
    OP = mybir.AluOpType
    from concourse import bass_isa
    from concourse.masks import make_identity
    RMAX = bass_isa.ReduceOp.max
    RADD = bass_isa.ReduceOp.add
    with (
        tc.tile_pool(name="wres", bufs=1) as wres,      # resident weights/tables
        tc.tile_pool(name="kv", bufs=1) as kvp,         # k1,k2,v resident
        tc.tile_pool(name="xp", bufs=NDT) as xp,        # x1 resident / x2 stream
        tc.tile_pool(name="stg", bufs=NTT) as stg,      # fp32 score staging
        tc.tile_pool(name="sml", bufs=2) as sml,        # small rotating scratch
        tc.tile_pool(name="att", bufs=2) as att,        # bf16 attn weights
        tc.tile_pool(name="sts", bufs=1) as sts,        # softmax stats
        tc.tile_pool(name="ao", bufs=4) as aop,         # attn out per head
        tc.tile_pool(name="ost", bufs=2) as ostp,       # out staging
        tc.tile_pool(name="pacc", bufs=3, space="PSUM") as pacc,
        tc.tile_pool(name="psc", bufs=2, space="PSUM") as pscp,
        tc.tile_pool(name="pav", bufs=1, space="PSUM") as pavp,
        tc.tile_pool(name="po", bufs=2, space="PSUM") as pop,
    ):
        # resident loads (partition-major dram layouts; all contiguous DMAs)
        wk1 = wres.tile([128, NDT, 128], BF)
        wk2 = wres.tile([128, NDT, 128], BF)
        wv = wres.tile([128, NDT, 128], BF)
        nc.scalar.dma_start(wk1[:], wk1_e[:].rearrange("p (a b) -> p a b", b=128))
        nc.scalar.dma_start(wk2[:], wk2_e[:].rearrange("p (a b) -> p a b", b=128))
        nc.scalar.dma_start(wv[:], wv_e[:].rearrange("p (a b) -> p a b", b=128))
        wq1m, wq2m = [], []
        for m in range(NM):
            w1 = wres.tile([128, NDT, 128], BF, name=f"wq1m{m}")
            w2 = wres.tile([128, NDT, 128], BF, name=f"wq2m{m}")
            nc.scalar.dma_start(w1[:], wq1_e[m].rearrange("p (a b) -> p a b", b=128))
            nc.scalar.dma_start(w2[:], wq2_e[m].rearrange("p (a b) -> p a b", b=128))
            wq1m.append(w1)
            wq2m.append(w2)
        tbl = wres.tile([128, S], F32)   # rows 0:64 cos, rows 64:128 sin
        masks = wres.tile([128, 4, 512], BF)
        ident = wres.tile([128, 128], BF)
        make_identity(nc, ident)
        nc.scalar.dma_start(tbl[:], cos_e[:])
        nc.scalar.dma_start(masks[:], msk_e[:].rearrange("p (a b) -> p a b", b=512))
        wo = wres.tile([128, NM, NDT, 128], BF)
        nc.scalar.dma_start(wo[:], wo_e[:].rearrange("p (a b c) -> p a b c",
                                                   b=NDT, c=128))

        k1 = kvp.tile([128, S], BF)
        k2 = kvp.tile([128, S], BF)
        vsb = kvp.tile([128, S], BF)   # [t_local, tt*128+e]

        def rope(psrc, cs):
            """rope a [128,512] psum tile -> fp32 sbuf tile (5 DVE ops)"""
            t1 = sml.tile([128, 512], F32, tag="t1")
            t2 = sml.tile([128, 512], F32, tag="t2")
            r = sml.tile([128, 512], F32, tag="rope")
            nc.vector.tensor_mul(t1[0:64, :], psrc[0:64, :], tbl[0:64, cs])
            nc.vector.tensor_mul(t1[64:128, :], psrc[64:128, :], tbl[0:64, cs])
            nc.vector.tensor_mul(t2[0:64, :], psrc[64:128, :], tbl[64:128, cs])
            nc.vector.tensor_mul(t2[64:128, :], psrc[0:64, :], tbl[64:128, cs])
            nc.vector.tensor_sub(r[0:64, :], t1[0:64, :], t2[0:64, :])
            nc.vector.tensor_add(r[64:128, :], t1[64:128, :], t2[64:128, :])
            return r

        for sb in range(NSB):
            cs = bass.ts(sb, 512)       # column slice for this seq block
            nt = 4 * (sb + 1)           # causal key tiles

            x1t = []
            for dt in range(NDT):
                a = xp.tile([128, 512], BF, tag="x1")
                nc.sync.dma_start(a[:], x1_e[sb, dt])
                x1t.append(a)

            def x2tile(dt):
                b = xp.tile([128, 512], BF, tag="x2", bufs=6)
                nc.sync.dma_start(b[:], x2_e[sb, dt])
                return b

            # ---- K (x1 passes) + vT in one dt pass; K x2 pass rides pair 0 ----
            pk = pacc.tile([128, 512], F32, tag="acc")
            pvT = pacc.tile([128, 512], F32, tag="acc")
            for dt in range(NDT):
                fst, lst = dt == 0, dt == NDT - 1
                nc.tensor.matmul(pk[:], wk1[:, dt], x1t[dt][:],
                                 start=fst, stop=False)
                nc.tensor.matmul(pk[:], wk2[:, dt], x1t[dt][:],
                                 start=False, stop=False)
                nc.tensor.matmul(pvT[:], wv[:, dt], x1t[dt][:],
                                 start=fst, stop=lst)

            vT_sb = sml.tile([128, 512], BF, tag="vts")
            nc.scalar.activation(vT_sb[:], pvT[:], AF.Copy)
            for tt in range(4):
                g = 4 * sb + tt
                tp = pscp.tile([128, 128], BF, tag="sc", name=f"tp{g}")
                nc.tensor.transpose(tp[:], vT_sb[:, bass.ts(tt, 128)], ident[:])
                nc.scalar.activation(vsb[:, bass.ts(g, 128)], tp[:], AF.Copy)
                nc.sync.dma_start(v_e[g], vsb[:, bass.ts(g, 128)])

            # ---- Q projection in head pairs + rope + scaled split ----
            q1h, q2h = [None] * NM, [None] * NM
            for pair in range(2):
                mA, mB = 2 * pair, 2 * pair + 1
                pqA = pacc.tile([128, 512], F32, tag="acc", name=f"pqA{pair}")
                pqB = pacc.tile([128, 512], F32, tag="acc", name=f"pqB{pair}")
                for dt in range(NDT):
                    fst = dt == 0
                    x2 = x2tile(dt)
                    if pair == 0:
                        nc.tensor.matmul(pk[:], wk1[:, dt], x2[:],
                                         start=False, stop=(dt == NDT - 1))
                    nc.tensor.matmul(pqA[:], wq1m[mA][:, dt], x2[:],
                                     start=fst, stop=False)
                    nc.tensor.matmul(pqB[:], wq1m[mB][:, dt], x2[:],
                                     start=fst, stop=False)
                if pair == 0:
                    kTs = rope(pk, cs)
                    nc.scalar.activation(k1[:, cs], kTs[:], AF.Copy)
                    k1f = sml.tile([128, 512], F32, tag="sub")
                    nc.scalar.activation(k1f[:], k1[:, cs], AF.Copy)
                    nc.vector.tensor_sub(k2[:, cs], kTs[:], k1f[:])
                    nc.sync.dma_start(kt_e[:, cs], kTs[:])
                for dt in range(NDT):
                    lst = dt == NDT - 1
                    nc.tensor.matmul(pqA[:], wq1m[mA][:, dt], x1t[dt][:],
                                     start=False, stop=False)
                    nc.tensor.matmul(pqA[:], wq2m[mA][:, dt], x1t[dt][:],
                                     start=False, stop=lst)
                for dt in range(NDT):
                    lst = dt == NDT - 1
                    nc.tensor.matmul(pqB[:], wq1m[mB][:, dt], x1t[dt][:],
                                     start=False, stop=False)
                    nc.tensor.matmul(pqB[:], wq2m[mB][:, dt], x1t[dt][:],
                                     start=False, stop=lst)
                for m, pq in ((mA, pqA), (mB, pqB)):
                    qTs = rope(pq, cs)
                    q1 = aop.tile([128, 512], BF, tag="q1")
                    q2 = aop.tile([128, 512], BF, tag="q2")
                    nc.scalar.activation(q1[:], qTs[:], AF.Copy, scale=SCALE)
                    q1f = sml.tile([128, 512], F32, tag="sub")
                    nc.scalar.activation(q1f[:], q1[:], AF.Copy)
                    nc.vector.scalar_tensor_tensor(q2[:], qTs[:], SCALE, q1f[:],
                                                   op0=OP.mult, op1=OP.subtract)
                    q1h[m], q2h[m] = q1, q2

            # ---- attention per head ----
            aout = []
            for h in range(NM):
                stgs = []
                amax = sts.tile([128, 512], F32, tag="amax")
                for tt in range(nt):
                    ks = bass.ts(tt, 128)
                    ps = pscp.tile([128, 512], F32, tag="sc")
                    nc.tensor.matmul(ps[:], k1[:, ks], q1h[h][:],
                                     start=True, stop=False)
                    nc.tensor.matmul(ps[:], k1[:, ks], q2h[h][:],
                                     start=False, stop=False)
                    nc.tensor.matmul(ps[:], k2[:, ks], q1h[h][:],
                                     start=False, stop=True)
                    st = stg.tile([128, 512], F32, tag="stage")
                    j = tt - 4 * sb
                    if j >= 0:
                        nc.vector.tensor_add(st[:], ps[:], masks[:, j])
                        src_fold = st
                    else:
                        nc.scalar.activation(st[:], ps[:], AF.Copy)
                        src_fold = ps
                    if tt == 0:
                        nc.vector.tensor_copy(amax[:], src_fold[:])
                    else:
                        nc.vector.tensor_max(amax[:], amax[:], src_fold[:])
                    stgs.append(st)
                mxb = sts.tile([128, 512], F32, tag="mxb")
                nc.gpsimd.partition_all_reduce(mxb[:], amax[:], channels=128,
                                               reduce_op=RMAX)
                pv2 = pavp.tile([128, 512], F32, tag="av")
                asum = sts.tile([128, 512], F32, tag="asum")
                for tt in range(nt):
                    sub = sml.tile([128, 512], F32, tag="sub")
                    nc.vector.tensor_sub(sub[:], stgs[tt][:], mxb[:])
                    at = att.tile([128, 512], BF, tag="at")
                    nc.scalar.activation(at[:], sub[:], AF.Exp)
                    if tt == 0:
                        nc.vector.tensor_copy(asum[:], at[:])
                    else:
                        nc.vector.tensor_add(asum[:], asum[:], at[:])
                    nc.tensor.matmul(pv2[:], vsb[:, bass.ts(tt, 128)], at[:],
                                     start=(tt == 0), stop=(tt == nt - 1))
                dsb = sts.tile([128, 512], F32, tag="dsb")
                rcb = sts.tile([128, 512], F32, tag="rcb")
                nc.gpsimd.partition_all_reduce(dsb[:], asum[:], channels=128,
                                               reduce_op=RADD)
                nc.vector.reciprocal(rcb[:], dsb[:])
                ah = aop.tile([128, 512], BF, tag="aout")
                nc.vector.tensor_mul(ah[:], pv2[:], rcb[:])
                aout.append(ah)

            # ---- O projection ----
            for dt in range(NDT):
                po = pop.tile([128, 512], F32, tag="o")
                for et in range(NM):
                    nc.tensor.matmul(po[:], wo[:, et, dt], aout[et][:],
                                     start=(et == 0), stop=(et == NM - 1))
                ost = ostp.tile([128, 512], F32, tag="ost")
                nc.scalar.activation(ost[:], po[:], AF.Copy)
                nc.sync.dma_start(out_e[sb, dt], ost[:])


def _prep_shards(x, position_ids, w_q, w_k, w_v, w_o):
    xs = np.ascontiguousarray(x.reshape(S, D_MODEL).T.astype(np.float32))
    x1 = xs.astype(BF16)
    x2 = (xs - x1.astype(np.float32)).astype(BF16)

    def xfmt(a):
        return np.ascontiguousarray(
            a.reshape(NDT, 128, NSB, 512).transpose(2, 0, 1, 3))

    x1s, x2s = xfmt(x1), xfmt(x2)

    pos = position_ids.astype(np.float32)
    inv = 1.0 / (THETA ** (np.arange(0, HD, 2, dtype=np.float32) / HD))  # 64
    ang = pos[None, :] * inv[:, None]          # [64, S]
    cosT = np.empty((128, S), np.float32)      # rows 0:64 cos, 64:128 sin
    cosT[0:64] = np.cos(ang)
    cosT[64:128] = np.sin(ang)

    masks = np.zeros((4, 128, 512), np.float32)  # cast to bf16 below
    jj = np.arange(128)
    cc = np.arange(512)
    for j in range(4):
        masks[j][(128 * j + jj)[:, None] > cc[None, :]] = NEG

    in_maps = []
    for i in range(NC_CORES):
        wq = np.ascontiguousarray(w_q[512 * i:512 * (i + 1)].T)   # [2560,512]
        wq1 = wq.astype(BF16)
        wq2 = (wq - wq1.astype(np.float32)).astype(BF16)

        def qfmt(a):
            # [2560, 512] -> [m, p, dt*128]: A[m, p, dt*128+c] = a[dt*128+p, m*128+c]
            return np.ascontiguousarray(
                a.reshape(NDT, 128, NM, 128).transpose(2, 1, 0, 3).reshape(
                    NM, 128, NDT * 128))

        def kfmt(a):
            # [2560, 128] -> [p, dt*128]: A[p, dt*128+c] = a[dt*128+p, c]
            return np.ascontiguousarray(
                a.reshape(NDT, 128, 128).transpose(1, 0, 2).reshape(128, NDT * 128))

        wk = np.ascontiguousarray(w_k[128 * i:128 * (i + 1)].T)   # [2560,128]
        wk1 = wk.astype(BF16)
        wk2 = (wk - wk1.astype(np.float32)).astype(BF16)
        wv = np.ascontiguousarray(
            w_v[128 * i:128 * (i + 1)].T).astype(BF16)
        wo = np.ascontiguousarray(w_o[:, 512 * i:512 * (i + 1)].T)  # [512,2560]
        # wo tile layout [p=e, (et, dt, c=d)]: A[e, et, dt, d] = wo[et*128+e, dt*128+d]
        wof = np.ascontiguousarray(
            wo.reshape(NM, 128, NDT, 128).transpose(1, 0, 2, 3).reshape(
                128, NM * NDT * 128)).astype(BF16)
        in_maps.append({
            "x1": x1s, "x2": x2s,
            "wq1": qfmt(wq1), "wq2": qfmt(wq2),
            "wk1": kfmt(wk1),
            "wk2": kfmt(wk2),
            "wv": kfmt(wv),
            "wo": wof,
            "cosT": cosT,
            "masks": masks.transpose(1, 0, 2).reshape(128, 4 * 512).astype(BF16),
        })
    return in_maps


def kernel(x, position_ids, w_q, w_k, w_v, w_o):
    x = np.asarray(x); position_ids = np.asarray(position_ids)
    w_q = np.asarray(w_q); w_k = np.asarray(w_k)
    w_v = np.asarray(w_v); w_o = np.asarray(w_o)

    if "nc" not in _cache:
        _cache["nc"] = _build()
    nc = _cache["nc"]

    in_maps = _prep_shards(x, position_ids, w_q, w_k, w_v, w_o)
    res = run_bass_kernel_spmd(nc, in_maps, core_ids=list(range(NC_CORES)))

    outT = np.zeros((NSB, NDT, 128, 512), np.float64)
    ks, vs = [], []
    for i in range(NC_CORES):
        r = res.results[i]
        outT += r["outT"].astype(np.float64)
        ks.append(np.ascontiguousarray(r["kT"].T))                 # [S, 128]
        vs.append(r["vout"].astype(np.float32).reshape(S, HD))
    out = outT.transpose(1, 2, 0, 3).reshape(D_MODEL, S).T.astype(np.float32)
    out = out.reshape(1, S, D_MODEL)
    k = np.stack(ks)[None].astype(np.float32)                      # [1,8,S,128]
    v = np.stack(vs)[None]
    return out, k, v


# revision 26
# speedup vs baseline: 1.0196x; 1.0196x over previous
"""GQA attention (32 Q heads / 8 KV heads, head_dim 128, d_model 2560, s=2048)
with RoPE, tensor-parallel across 8 TRN2 NeuronCores.

Sharding: core i owns Q heads 4i..4i+3 and KV head i (w_q/w_k/w_v sharded on
the head output dim, w_o on its input dim). Each core produces a full-shape
partial of the output projection; the partials are summed on the host (the
"all-reduce after w_o" of the hint, done at unshard time), so no on-device
collective is needed and outputs (k/v caches, out rows) are disjoint/partial.

Precision: the softmax logits here have std ~2.5e3 (softmax is near one-hot),
so Q/K projections and QK^T are computed as 3-pass split-bf16 matmuls
(a1b1+a1b2+a2b1, fp32-equivalent to ~2^-18) while the V chain (V proj,
attn@V, O proj) runs in plain bf16. Measured end-to-end rel err ~3e-3.

Layout: everything runs transposed ([dim, seq] on chip) so that projections,
scores^T, attn@V and the O projection all chain without any transposes.
Softmax runs over the partition axis via gpsimd C-axis reduce + broadcast.
"""
import sys
sys.path.insert(0, '/opt/trn_rl_repo')

import numpy as np
import ml_dtypes

import concourse.bass as bass
import concourse.tile as tile
from concourse import bacc, mybir
from concourse.bass_utils import run_bass_kernel_spmd

BF16 = ml_dtypes.bfloat16
F32 = mybir.dt.float32
BF = mybir.dt.bfloat16

D_MODEL = 2560
NUM_HEADS = 32
NUM_KV = 8
HD = 128
S = 2048
THETA = 5000000.0
NC_CORES = 8
NDT = D_MODEL // 128      # 20 contraction tiles
NSB = S // 512            # 4 sequence blocks of 512
NTT = S // 128            # 16 key tiles of 128
NM = 4                    # q heads per core
SCALE = 1.0 / np.sqrt(HD)
NEG = -1.0e30

_cache = {}


def _build():
    nc = bacc.Bacc("TRN2", target_bir_lowering=False, debug=False,
                   num_devices=NC_CORES)
    dp = nc.declare_dram_parameter
    x1_e = dp("x1", [NSB, NDT, 128, 512], BF, isOutput=False)
    x2_e = dp("x2", [NSB, NDT, 128, 512], BF, isOutput=False)
    wq1_e = dp("wq1", [NM, 128, NDT * 128], BF, isOutput=False)
    wq2_e = dp("wq2", [NM, 128, NDT * 128], BF, isOutput=False)
    wk1_e = dp("wk1", [128, NDT * 128], BF, isOutput=False)
    wk2_e = dp("wk2", [128, NDT * 128], BF, isOutput=False)
    wv_e = dp("wv", [128, NDT * 128], BF, isOutput=False)
    wo_e = dp("wo", [128, NM * NDT * 128], BF, isOutput=False)
    cos_e = dp("cosT", [128, S], F32, isOutput=False)  # rows 0:64 cos, 64:128 sin
    msk_e = dp("masks", [128, 4 * 512], BF, isOutput=False)
    out_e = dp("outT", [NSB, NDT, 128, 512], F32, isOutput=True)
    kt_e = dp("kT", [128, S], F32, isOutput=True)
    v_e = dp("vout", [NTT, 128, 128], BF, isOutput=True)

    with tile.TileContext(nc) as tc:
        _body(nc, tc, x1_e, x2_e, wq1_e, wq2_e, wk1_e, wk2_e, wv_e, wo_e,
              cos_e, msk_e, out_e, kt_e, v_e)
    nc.compile()
    return nc


def _body(nc, tc, x1_e, x2_e, wq1_e, wq2_e, wk1_e, wk2_e, wv_e, wo_e,
          cos_e, msk_e, out_e, kt_e, v_e):
    AF = mybir.ActivationFunctionType
    OP = mybir.AluOpType
    from concourse import bass_isa
    from concourse.masks import make_identity
    RMAX = bass_isa.ReduceOp.max
    RADD = bass_isa.ReduceOp.add
    with (
        tc.tile_pool(name="wres", bufs=1) as wres,      # resident weights/tables
        tc.tile_pool(name="kv", bufs=1) as kvp,         # k1,k2,v resident
        tc.tile_pool(name="xp", bufs=NDT) as xp,        # x1 resident / x2 stream
        tc.tile_pool(name="stg", bufs=1) as stg,      # fp32 score staging
        tc.tile_pool(name="sml", bufs=2) as sml,        # small rotating scratch
        tc.tile_pool(name="att", bufs=2) as att,        # bf16 attn weights
        tc.tile_pool(name="sts", bufs=1) as sts,        # softmax stats
        tc.tile_pool(name="ao", bufs=4) as aop,         # attn out per head
        tc.tile_pool(name="ost", bufs=2) as ostp,       # out staging
        tc.tile_pool(name="pacc", bufs=3, space="PSUM") as pacc,
        tc.tile_pool(name="psc", bufs=2, space="PSUM") as pscp,
        tc.tile_pool(name="pav", bufs=1, space="PSUM") as pavp,
        tc.tile_pool(name="po", bufs=2, space="PSUM") as pop,
    ):
        # resident loads (partition-major dram layouts; all contiguous DMAs)
        wk1 = wres.tile([128, NDT, 128], BF)
        wk2 = wres.tile([128, NDT, 128], BF)
        wv = wres.tile([128, NDT, 128], BF)
        nc.scalar.dma_start(wk1[:], wk1_e[:].rearrange("p (a b) -> p a b", b=128))
        nc.scalar.dma_start(wk2[:], wk2_e[:].rearrange("p (a b) -> p a b", b=128))
        nc.scalar.dma_start(wv[:], wv_e[:].rearrange("p (a b) -> p a b", b=128))
        wq1m, wq2m = [], []
        for m in range(NM):
            w1 = wres.tile([128, NDT, 128], BF, name=f"wq1m{m}")
            w2 = wres.tile([128, NDT, 128], BF, name=f"wq2m{m}")
            nc.scalar.dma_start(w1[:], wq1_e[m].rearrange("p (a b) -> p a b", b=128))
            nc.scalar.dma_start(w2[:], wq2_e[m].rearrange("p (a b) -> p a b", b=128))
            wq1m.append(w1)
            wq2m.append(w2)
        tbl = wres.tile([128, S], F32)   # rows 0:64 cos, rows 64:128 sin
        masks = wres.tile([128, 4, 512], BF)
        ident = wres.tile([128, 128], BF)
        make_identity(nc, ident)
        nc.scalar.dma_start(tbl[:], cos_e[:])
        nc.scalar.dma_start(masks[:], msk_e[:].rearrange("p (a b) -> p a b", b=512))
        wo = wres.tile([128, NM, NDT, 128], BF)
        nc.scalar.dma_start(wo[:], wo_e[:].rearrange("p (a b c) -> p a b c",
                                                   b=NDT, c=128))

        stgbig = stg.tile([128, NTT * 512], F32)   # score staging, slice per key tile
        k1 = kvp.tile([128, S], BF)
        k2 = kvp.tile([128, S], BF)
        vsb = kvp.tile([128, S], BF)   # [t_local, tt*128+e]

        def rope(psrc, cs):
            """rope a [128,512] psum tile -> fp32 sbuf tile (5 DVE ops)"""
            t1 = sml.tile([128, 512], F32, tag="t1")
            t2 = sml.tile([128, 512], F32, tag="t2")
            r = sml.tile([128, 512], F32, tag="rope")
            nc.vector.tensor_mul(t1[0:64, :], psrc[0:64, :], tbl[0:64, cs])
            nc.vector.tensor_mul(t1[64:128, :], psrc[64:128, :], tbl[0:64, cs])
            nc.vector.tensor_mul(t2[0:64, :], psrc[64:128, :], tbl[64:128, cs])
            nc.vector.tensor_mul(t2[64:128, :], psrc[0:64, :], tbl[64:128, cs])
            nc.vector.tensor_sub(r[0:64, :], t1[0:64, :], t2[0:64, :])
            nc.vector.tensor_add(r[64:128, :], t1[64:128, :], t2[64:128, :])
            return r

        for sb in range(NSB):
            cs = bass.ts(sb, 512)       # column slice for this seq block
            nt = 4 * (sb + 1)           # causal key tiles

            x1t = []
            for dt in range(NDT):
                a = xp.tile([128, 512], BF, tag="x1")
                nc.sync.dma_start(a[:], x1_e[sb, dt])
                x1t.append(a)

            def x2tile(dt):
                b = xp.tile([128, 512], BF, tag="x2", bufs=6)
                nc.sync.dma_start(b[:], x2_e[sb, dt])
                return b

            # ---- K (x1 passes) + vT in one dt pass; K x2 pass rides pair 0 ----
            pk = pacc.tile([128, 512], F32, tag="acc")
            pvT = pacc.tile([128, 512], F32, tag="acc")
            for dt in range(NDT):
                fst, lst = dt == 0, dt == NDT - 1
                nc.tensor.matmul(pk[:], wk1[:, dt], x1t[dt][:],
                                 start=fst, stop=False)
                nc.tensor.matmul(pk[:], wk2[:, dt], x1t[dt][:],
                                 start=False, stop=False)
                nc.tensor.matmul(pvT[:], wv[:, dt], x1t[dt][:],
                                 start=fst, stop=lst)

            vT_sb = sml.tile([128, 512], BF, tag="vts")
            nc.scalar.activation(vT_sb[:], pvT[:], AF.Copy)
            for tt in range(4):
                g = 4 * sb + tt
                tp = pscp.tile([128, 128], BF, tag="sc", name=f"tp{g}")
                nc.tensor.transpose(tp[:], vT_sb[:, bass.ts(tt, 128)], ident[:])
                nc.scalar.activation(vsb[:, bass.ts(g, 128)], tp[:], AF.Copy)
                nc.sync.dma_start(v_e[g], vsb[:, bass.ts(g, 128)])

            # ---- Q projection in head pairs + rope + scaled split ----
            q1h, q2h = [None] * NM, [None] * NM
            for pair in range(2):
                mA, mB = 2 * pair, 2 * pair + 1
                pqA = pacc.tile([128, 512], F32, tag="acc", name=f"pqA{pair}")
                pqB = pacc.tile([128, 512], F32, tag="acc", name=f"pqB{pair}")
                for dt in range(NDT):
                    fst = dt == 0
                    x2 = x2tile(dt)
                    if pair == 0:
                        nc.tensor.matmul(pk[:], wk1[:, dt], x2[:],
                                         start=False, stop=(dt == NDT - 1))
                    nc.tensor.matmul(pqA[:], wq1m[mA][:, dt], x2[:],
                                     start=fst, stop=False)
                    nc.tensor.matmul(pqB[:], wq1m[mB][:, dt], x2[:],
                                     start=fst, stop=False)
                if pair == 0:
                    kTs = rope(pk, cs)
                    nc.scalar.activation(k1[:, cs], kTs[:], AF.Copy)
                    k1f = sml.tile([128, 512], F32, tag="sub")
                    nc.scalar.activation(k1f[:], k1[:, cs], AF.Copy)
                    nc.vector.tensor_sub(k2[:, cs], kTs[:], k1f[:])
                    nc.sync.dma_start(kt_e[:, cs], kTs[:])
                for dt in range(NDT):
                    lst = dt == NDT - 1
                    nc.tensor.matmul(pqA[:], wq1m[mA][:, dt], x1t[dt][:],
                                     start=False, stop=False)
                    nc.tensor.matmul(pqA[:], wq2m[mA][:, dt], x1t[dt][:],
                                     start=False, stop=lst)
                for dt in range(NDT):
                    lst = dt == NDT - 1
                    nc.tensor.matmul(pqB[:], wq1m[mB][:, dt], x1t[dt][:],
                                     start=False, stop=False)
                    nc.tensor.matmul(pqB[:], wq2m[mB][:, dt], x1t[dt][:],
                                     start=False, stop=lst)
                for m, pq in ((mA, pqA), (mB, pqB)):
                    qTs = rope(pq, cs)
                    q1 = aop.tile([128, 512], BF, tag="q1")
                    q2 = aop.tile([128, 512], BF, tag="q2")
                    nc.scalar.activation(q1[:], qTs[:], AF.Copy, scale=SCALE)
                    q1f = sml.tile([128, 512], F32, tag="sub")
                    nc.scalar.activation(q1f[:], q1[:], AF.Copy)
                    nc.vector.scalar_tensor_tensor(q2[:], qTs[:], SCALE, q1f[:],
                                                   op0=OP.mult, op1=OP.subtract)
                    q1h[m], q2h[m] = q1, q2

            # ---- attention per head ----
            aout = []
            for h in range(NM):
                amax = sts.tile([128, 512], F32, tag="amax")
                for tt in range(nt):
                    ks = bass.ts(tt, 128)
                    ps = pscp.tile([128, 512], F32, tag="sc")
                    nc.tensor.matmul(ps[:], k1[:, ks], q1h[h][:],
                                     start=True, stop=False)
                    nc.tensor.matmul(ps[:], k1[:, ks], q2h[h][:],
                                     start=False, stop=False)
                    nc.tensor.matmul(ps[:], k2[:, ks], q1h[h][:],
                                     start=False, stop=True)
                    st = stgbig[:, bass.ts(tt, 512)]
                    j = tt - 4 * sb
                    if j >= 0:
                        nc.vector.tensor_add(st, ps[:], masks[:, j])
                    else:
                        nc.scalar.activation(st, ps[:], AF.Copy)
                # one strided reduce over all key tiles: innermost dim = tt
                stv = stgbig[:, 0:nt * 512].rearrange("p (t c) -> p c t", c=512)
                nc.vector.tensor_reduce(amax[:], stv, axis=mybir.AxisListType.X,
                                        op=OP.max)
                mxb = sts.tile([128, 512], F32, tag="mxb")
                nc.gpsimd.partition_all_reduce(mxb[:], amax[:], channels=128,
                                               reduce_op=RMAX)
                # one broadcast subtract over the whole staged block (in place)
                blk = stgbig[:, 0:nt * 512].rearrange("p (t c) -> p t c", c=512)
                nc.vector.tensor_sub(blk, blk,
                                     mxb[:].unsqueeze(1).broadcast_to([128, nt, 512]))
                pv2 = pavp.tile([128, 512], F32, tag="av")
                asum = sts.tile([128, 512], F32, tag="asum")
                for tt in range(nt):
                    at = att.tile([128, 512], BF, tag="at")
                    nc.scalar.activation(at[:], stgbig[:, bass.ts(tt, 512)],
                                         AF.Exp)
                    if tt == 0:
                        nc.vector.tensor_copy(asum[:], at[:])
                    else:
                        nc.vector.tensor_add(asum[:], asum[:], at[:])
                    nc.tensor.matmul(pv2[:], vsb[:, bass.ts(tt, 128)], at[:],
                                     start=(tt == 0), stop=(tt == nt - 1))
                dsb = sts.tile([128, 512], F32, tag="dsb")
                rcb = sts.tile([128, 512], F32, tag="rcb")
                nc.gpsimd.partition_all_reduce(dsb[:], asum[:], channels=128,
                                               reduce_op=RADD)
                nc.vector.reciprocal(rcb[:], dsb[:])
                ah = aop.tile([128, 512], BF, tag="aout")
                nc.vector.tensor_mul(ah[:], pv2[:], rcb[:])
                aout.append(ah)

            # ---- O projection ----
            for dt in range(NDT):
                po = pop.tile([128, 512], F32, tag="o")
                for et in range(NM):
                    nc.tensor.matmul(po[:], wo[:, et, dt], aout[et][:],
                                     start=(et == 0), stop=(et == NM - 1))
                ost = ostp.tile([128, 512], F32, tag="ost")
                nc.scalar.activation(ost[:], po[:], AF.Copy)
                nc.sync.dma_start(out_e[sb, dt], ost[:])


def _prep_shards(x, position_ids, w_q, w_k, w_v, w_o):
    xs = np.ascontiguousarray(x.reshape(S, D_MODEL).T.astype(np.float32))
    x1 = xs.astype(BF16)
    x2 = (xs - x1.astype(np.float32)).astype(BF16)

    def xfmt(a):
        return np.ascontiguousarray(
            a.reshape(NDT, 128, NSB, 512).transpose(2, 0, 1, 3))

    x1s, x2s = xfmt(x1), xfmt(x2)

    pos = position_ids.astype(np.float32)
    inv = 1.0 / (THETA ** (np.arange(0, HD, 2, dtype=np.float32) / HD))  # 64
    ang = pos[None, :] * inv[:, None]          # [64, S]
    cosT = np.empty((128, S), np.float32)      # rows 0:64 cos, 64:128 sin
    cosT[0:64] = np.cos(ang)
    cosT[64:128] = np.sin(ang)

    masks = np.zeros((4, 128, 512), np.float32)  # cast to bf16 below
    jj = np.arange(128)
    cc = np.arange(512)
    for j in range(4):
        masks[j][(128 * j + jj)[:, None] > cc[None, :]] = NEG

    in_maps = []
    for i in range(NC_CORES):
        wq = np.ascontiguousarray(w_q[512 * i:512 * (i + 1)].T)   # [2560,512]
        wq1 = wq.astype(BF16)
        wq2 = (wq - wq1.astype(np.float32)).astype(BF16)

        def qfmt(a):
            # [2560, 512] -> [m, p, dt*128]: A[m, p, dt*128+c] = a[dt*128+p, m*128+c]
            return np.ascontiguousarray(
                a.reshape(NDT, 128, NM, 128).transpose(2, 1, 0, 3).reshape(
                    NM, 128, NDT * 128))

        def kfmt(a):
            # [2560, 128] -> [p, dt*128]: A[p, dt*128+c] = a[dt*128+p, c]
            return np.ascontiguousarray(
                a.reshape(NDT, 128, 128).transpose(1, 0, 2).reshape(128, NDT * 128))

        wk = np.ascontiguousarray(w_k[128 * i:128 * (i + 1)].T)   # [2560,128]
        wk1 = wk.astype(BF16)
        wk2 = (wk - wk1.astype(np.float32)).astype(BF16)
        wv = np.ascontiguousarray(
            w_v[128 * i:128 * (i + 1)].T).astype(BF16)
        wo = np.ascontiguousarray(w_o[:, 512 * i:512 * (i + 1)].T)  # [512,2560]
        # wo tile layout [p=e, (et, dt, c=d)]: A[e, et, dt, d] = wo[et*128+e, dt*128+d]
        wof = np.ascontiguousarray(
            wo.reshape(NM, 128, NDT, 128).transpose(1, 0, 2, 3).reshape(
                128, NM * NDT * 128)).astype(BF16)
        in_maps.append({
            "x1": x1s, "x2": x2s,
            "wq1": qfmt(wq1), "wq2": qfmt(wq2),
            "wk1": kfmt(wk1),
            "wk2": kfmt(wk2),
            "wv": kfmt(wv),
            "wo": wof,
            "cosT": cosT,
            "masks": masks.transpose(1, 0, 2).reshape(128, 4 * 512).astype(BF16),
        })
    return in_maps


def kernel(x, position_ids, w_q, w_k, w_v, w_o):
    x = np.asarray(x); position_ids = np.asarray(position_ids)
    w_q = np.asarray(w_q); w_k = np.asarray(w_k)
    w_v = np.asarray(w_v); w_o = np.asarray(w_o)

    if "nc" not in _cache:
        _cache["nc"] = _build()
    nc = _cache["nc"]

    in_maps = _prep_shards(x, position_ids, w_q, w_k, w_v, w_o)
    res = run_bass_kernel_spmd(nc, in_maps, core_ids=list(range(NC_CORES)))

    outT = np.zeros((NSB, NDT, 128, 512), np.float64)
    ks, vs = [], []
    for i in range(NC_CORES):
        r = res.results[i]
        outT += r["outT"].astype(np.float64)
        ks.append(np.ascontiguousarray(r["kT"].T))                 # [S, 128]
        vs.append(r["vout"].astype(np.float32).reshape(S, HD))
    out = outT.transpose(1, 2, 0, 3).reshape(D_MODEL, S).T.astype(np.float32)
    out = out.reshape(1, S, D_MODEL)
    k = np.stack(ks)[None].astype(np.float32)                      # [1,8,S,128]
    v = np.stack(vs)[None]
    return out, k, v


# revision 30
# speedup vs baseline: 1.0425x; 1.0225x over previous
"""GQA attention (32 Q heads / 8 KV heads, head_dim 128, d_model 2560, s=2048)
with RoPE, tensor-parallel across 8 TRN2 NeuronCores.

Sharding: core i owns Q heads 4i..4i+3 and KV head i (w_q/w_k/w_v sharded on
the head output dim, w_o on its input dim). Each core produces a full-shape
partial of the output projection; the partials are summed on the host (the
"all-reduce after w_o" of the hint, done at unshard time), so no on-device
collective is needed and outputs (k/v caches, out rows) are disjoint/partial.

Precision: the softmax logits here have std ~2.5e3 (softmax is near one-hot),
so Q/K projections and QK^T are computed as 3-pass split-bf16 matmuls
(a1b1+a1b2+a2b1, fp32-equivalent to ~2^-18) while the V chain (V proj,
attn@V, O proj) runs in plain bf16. Measured end-to-end rel err ~3e-3.

Layout: everything runs transposed ([dim, seq] on chip) so that projections,
scores^T, attn@V and the O projection all chain without any transposes.
Softmax runs over the partition axis via gpsimd C-axis reduce + broadcast.
"""
import sys
sys.path.insert(0, '/opt/trn_rl_repo')

import numpy as np
import ml_dtypes

import concourse.bass as bass
import concourse.tile as tile
from concourse import bacc, mybir
from concourse.bass_utils import run_bass_kernel_spmd

BF16 = ml_dtypes.bfloat16
F32 = mybir.dt.float32
BF = mybir.dt.bfloat16

D_MODEL = 2560
NUM_HEADS = 32
NUM_KV = 8
HD = 128
S = 2048
THETA = 5000000.0
NC_CORES = 8
NDT = D_MODEL // 128      # 20 contraction tiles
NSB = S // 512            # 4 sequence blocks of 512
NTT = S // 128            # 16 key tiles of 128
NM = 4                    # q heads per core
SCALE = 1.0 / np.sqrt(HD)
NEG = -1.0e30

_cache = {}


def _build():
    nc = bacc.Bacc("TRN2", target_bir_lowering=False, debug=False,
                   num_devices=NC_CORES)
    dp = nc.declare_dram_parameter
    x1_e = dp("x1", [NSB, NDT, 128, 512], BF, isOutput=False)
    x2_e = dp("x2", [NSB, NDT, 128, 512], BF, isOutput=False)
    wq1_e = dp("wq1", [NM, 128, NDT * 128], BF, isOutput=False)
    wq2_e = dp("wq2", [NM, 128, NDT * 128], BF, isOutput=False)
    wk1_e = dp("wk1", [128, NDT * 128], BF, isOutput=False)
    wk2_e = dp("wk2", [128, NDT * 128], BF, isOutput=False)
    wv_e = dp("wv", [128, NDT * 128], BF, isOutput=False)
    wo_e = dp("wo", [128, NM * NDT * 128], BF, isOutput=False)
    cos_e = dp("cosT", [128, S], F32, isOutput=False)  # rows 0:64 cos, 64:128 sin
    msk_e = dp("masks", [128, 4 * 512], BF, isOutput=False)
    out_e = dp("outT", [NSB, NDT, 128, 512], F32, isOutput=True)
    kt_e = dp("kT", [128, S], F32, isOutput=True)
    v_e = dp("vout", [NTT, 128, 128], BF, isOutput=True)

    with tile.TileContext(nc) as tc:
        _body(nc, tc, x1_e, x2_e, wq1_e, wq2_e, wk1_e, wk2_e, wv_e, wo_e,
              cos_e, msk_e, out_e, kt_e, v_e)
    nc.compile()
    return nc


def _body(nc, tc, x1_e, x2_e, wq1_e, wq2_e, wk1_e, wk2_e, wv_e, wo_e,
          cos_e, msk_e, out_e, kt_e, v_e):
    AF = mybir.ActivationFunctionType
    OP = mybir.AluOpType
    from concourse import bass_isa
    from concourse.masks import make_identity
    RMAX = bass_isa.ReduceOp.max
    RADD = bass_isa.ReduceOp.add
    with (
        tc.tile_pool(name="wres", bufs=1) as wres,      # resident weights/tables
        tc.tile_pool(name="kv", bufs=1) as kvp,         # k1,k2,v resident
        tc.tile_pool(name="xp", bufs=NDT) as xp,        # x1 resident / x2 stream
        tc.tile_pool(name="stg", bufs=1) as stg,      # fp32 score staging
        tc.tile_pool(name="sml", bufs=2) as sml,        # small rotating scratch
        tc.tile_pool(name="att", bufs=2) as att,        # bf16 attn weights
        tc.tile_pool(name="sts", bufs=1) as sts,        # softmax stats
        tc.tile_pool(name="ao", bufs=4) as aop,         # attn out per head
        tc.tile_pool(name="ost", bufs=2) as ostp,       # out staging
        tc.tile_pool(name="pacc", bufs=3, space="PSUM") as pacc,
        tc.tile_pool(name="psc", bufs=2, space="PSUM") as pscp,
        tc.tile_pool(name="pav", bufs=1, space="PSUM") as pavp,
        tc.tile_pool(name="po", bufs=2, space="PSUM") as pop,
    ):
        # resident loads (partition-major dram layouts; all contiguous DMAs)
        wk1 = wres.tile([128, NDT, 128], BF)
        wk2 = wres.tile([128, NDT, 128], BF)
        wv = wres.tile([128, NDT, 128], BF)
        nc.scalar.dma_start(wk1[:], wk1_e[:].rearrange("p (a b) -> p a b", b=128))
        nc.scalar.dma_start(wk2[:], wk2_e[:].rearrange("p (a b) -> p a b", b=128))
        nc.scalar.dma_start(wv[:], wv_e[:].rearrange("p (a b) -> p a b", b=128))
        wq1m, wq2m = [], []
        for m in range(NM):
            w1 = wres.tile([128, NDT, 128], BF, name=f"wq1m{m}")
            w2 = wres.tile([128, NDT, 128], BF, name=f"wq2m{m}")
            nc.scalar.dma_start(w1[:], wq1_e[m].rearrange("p (a b) -> p a b", b=128))
            nc.scalar.dma_start(w2[:], wq2_e[m].rearrange("p (a b) -> p a b", b=128))
            wq1m.append(w1)
            wq2m.append(w2)
        tbl = wres.tile([128, S], F32)   # rows 0:64 cos, rows 64:128 sin
        masks = wres.tile([128, 4, 512], BF)
        ident = wres.tile([128, 128], BF)
        make_identity(nc, ident)
        nc.scalar.dma_start(tbl[:], cos_e[:])
        nc.scalar.dma_start(masks[:], msk_e[:].rearrange("p (a b) -> p a b", b=512))
        wo = wres.tile([128, NM, NDT, 128], BF)
        nc.scalar.dma_start(wo[:], wo_e[:].rearrange("p (a b c) -> p a b c",
                                                   b=NDT, c=128))

        stgA = stg.tile([128, 8 * 512], F32)   # score staging, key tiles 0..7
        stgB = stg.tile([128, 8 * 512], F32)   # score staging, key tiles 8..15
        k1 = kvp.tile([128, S], BF)
        k2 = kvp.tile([128, S], BF)
        vsb = kvp.tile([128, S], BF)   # [t_local, tt*128+e]

        def rope(psrc, cs):
            """rope a [128,512] psum tile -> fp32 sbuf tile (5 DVE ops)"""
            t1 = sml.tile([128, 512], F32, tag="t1")
            t2 = sml.tile([128, 512], F32, tag="t2")
            r = sml.tile([128, 512], F32, tag="rope")
            nc.vector.tensor_mul(t1[0:64, :], psrc[0:64, :], tbl[0:64, cs])
            nc.vector.tensor_mul(t1[64:128, :], psrc[64:128, :], tbl[0:64, cs])
            nc.vector.tensor_mul(t2[0:64, :], psrc[64:128, :], tbl[64:128, cs])
            nc.vector.tensor_mul(t2[64:128, :], psrc[0:64, :], tbl[64:128, cs])
            nc.vector.tensor_sub(r[0:64, :], t1[0:64, :], t2[0:64, :])
            nc.vector.tensor_add(r[64:128, :], t1[64:128, :], t2[64:128, :])
            return r

        for sb in range(NSB):
            cs = bass.ts(sb, 512)       # column slice for this seq block
            nt = 4 * (sb + 1)           # causal key tiles

            x1t = []
            for dt in range(NDT):
                a = xp.tile([128, 512], BF, tag="x1")
                nc.sync.dma_start(a[:], x1_e[sb, dt])
                x1t.append(a)

            def x2tile(dt):
                b = xp.tile([128, 512], BF, tag="x2", bufs=6)
                nc.sync.dma_start(b[:], x2_e[sb, dt])
                return b

            # ---- K (x1 passes) + vT in one dt pass; K x2 pass rides pair 0 ----
            pk = pacc.tile([128, 512], F32, tag="acc")
            pvT = pacc.tile([128, 512], F32, tag="acc")
            for dt in range(NDT):
                fst, lst = dt == 0, dt == NDT - 1
                nc.tensor.matmul(pk[:], wk1[:, dt], x1t[dt][:],
                                 start=fst, stop=False)
                nc.tensor.matmul(pk[:], wk2[:, dt], x1t[dt][:],
                                 start=False, stop=False)
                nc.tensor.matmul(pvT[:], wv[:, dt], x1t[dt][:],
                                 start=fst, stop=lst)

            vT_sb = sml.tile([128, 512], BF, tag="vts")
            nc.scalar.activation(vT_sb[:], pvT[:], AF.Copy)
            for tt in range(4):
                g = 4 * sb + tt
                tp = pscp.tile([128, 128], BF, tag="sc", name=f"tp{g}")
                nc.tensor.transpose(tp[:], vT_sb[:, bass.ts(tt, 128)], ident[:])
                nc.scalar.activation(vsb[:, bass.ts(g, 128)], tp[:], AF.Copy)
                nc.sync.dma_start(v_e[g], vsb[:, bass.ts(g, 128)])

            # ---- Q projection in head pairs + rope + scaled split ----
            q1h, q2h = [None] * NM, [None] * NM
            for pair in range(2):
                mA, mB = 2 * pair, 2 * pair + 1
                pqA = pacc.tile([128, 512], F32, tag="acc", name=f"pqA{pair}")
                pqB = pacc.tile([128, 512], F32, tag="acc", name=f"pqB{pair}")
                for dt in range(NDT):
                    fst = dt == 0
                    x2 = x2tile(dt)
                    if pair == 0:
                        nc.tensor.matmul(pk[:], wk1[:, dt], x2[:],
                                         start=False, stop=(dt == NDT - 1))
                    nc.tensor.matmul(pqA[:], wq1m[mA][:, dt], x2[:],
                                     start=fst, stop=False)
                    nc.tensor.matmul(pqB[:], wq1m[mB][:, dt], x2[:],
                                     start=fst, stop=False)
                if pair == 0:
                    kTs = rope(pk, cs)
                    nc.scalar.activation(k1[:, cs], kTs[:], AF.Copy)
                    k1f = sml.tile([128, 512], F32, tag="sub")
                    nc.scalar.activation(k1f[:], k1[:, cs], AF.Copy)
                    nc.vector.tensor_sub(k2[:, cs], kTs[:], k1f[:])
                    nc.sync.dma_start(kt_e[:, cs], kTs[:])
                for dt in range(NDT):
                    lst = dt == NDT - 1
                    nc.tensor.matmul(pqA[:], wq1m[mA][:, dt], x1t[dt][:],
                                     start=False, stop=False)
                    nc.tensor.matmul(pqA[:], wq2m[mA][:, dt], x1t[dt][:],
                                     start=False, stop=lst)
                for dt in range(NDT):
                    lst = dt == NDT - 1
                    nc.tensor.matmul(pqB[:], wq1m[mB][:, dt], x1t[dt][:],
                                     start=False, stop=False)
                    nc.tensor.matmul(pqB[:], wq2m[mB][:, dt], x1t[dt][:],
                                     start=False, stop=lst)
                for m, pq in ((mA, pqA), (mB, pqB)):
                    qTs = rope(pq, cs)
                    q1 = aop.tile([128, 512], BF, tag="q1")
                    q2 = aop.tile([128, 512], BF, tag="q2")
                    nc.scalar.activation(q1[:], qTs[:], AF.Copy, scale=SCALE)
                    q1f = sml.tile([128, 512], F32, tag="sub")
                    nc.scalar.activation(q1f[:], q1[:], AF.Copy)
                    nc.vector.scalar_tensor_tensor(q2[:], qTs[:], SCALE, q1f[:],
                                                   op0=OP.mult, op1=OP.subtract)
                    q1h[m], q2h[m] = q1, q2

            # ---- attention per head ----
            aout = []
            for h in range(NM):
                amax = sts.tile([128, 512], F32, tag="amax")
                for tt in range(nt):
                    ks = bass.ts(tt, 128)
                    ps = pscp.tile([128, 512], F32, tag="sc")
                    nc.tensor.matmul(ps[:], k1[:, ks], q1h[h][:],
                                     start=True, stop=False)
                    nc.tensor.matmul(ps[:], k1[:, ks], q2h[h][:],
                                     start=False, stop=False)
                    nc.tensor.matmul(ps[:], k2[:, ks], q1h[h][:],
                                     start=False, stop=True)
                    st = (stgA if tt < 8 else stgB)[:, bass.ts(tt % 8, 512)]
                    j = tt - 4 * sb
                    if j >= 0:
                        nc.vector.tensor_add(st, ps[:], masks[:, j])
                    else:
                        nc.scalar.activation(st, ps[:], AF.Copy)
                # strided reduce per staging half: innermost dim = tt
                na = min(nt, 8)
                sva = stgA[:, 0:na * 512].rearrange("p (t c) -> p c t", c=512)
                nc.vector.tensor_reduce(amax[:], sva, axis=mybir.AxisListType.X,
                                        op=OP.max)
                if nt > 8:
                    amaxB = sts.tile([128, 512], F32, tag="amaxB")
                    svb = stgB[:, 0:(nt - 8) * 512].rearrange(
                        "p (t c) -> p c t", c=512)
                    nc.vector.tensor_reduce(amaxB[:], svb,
                                            axis=mybir.AxisListType.X, op=OP.max)
                    nc.vector.tensor_max(amax[:], amax[:], amaxB[:])
                mxb = sts.tile([128, 512], F32, tag="mxb")
                nc.gpsimd.partition_all_reduce(mxb[:], amax[:], channels=128,
                                               reduce_op=RMAX)
                pv2 = pavp.tile([128, 512], F32, tag="av")
                asum = sts.tile([128, 512], F32, tag="asum")
                blkA = stgA[:, 0:na * 512].rearrange("p (t c) -> p t c", c=512)
                nc.vector.tensor_sub(blkA, blkA,
                                     mxb[:].unsqueeze(1).broadcast_to([128, na, 512]))
                if nt > 8:
                    blkB = stgB[:, 0:(nt - 8) * 512].rearrange(
                        "p (t c) -> p t c", c=512)
                    nc.vector.tensor_sub(
                        blkB, blkB,
                        mxb[:].unsqueeze(1).broadcast_to([128, nt - 8, 512]))
                for tt in range(nt):
                    sl = (stgA if tt < 8 else stgB)[:, bass.ts(tt % 8, 512)]
                    at = att.tile([128, 512], BF, tag="at")
                    nc.scalar.activation(at[:], sl, AF.Exp)
                    if tt == 0:
                        nc.vector.tensor_copy(asum[:], at[:])
                    else:
                        nc.vector.tensor_add(asum[:], asum[:], at[:])
                    nc.tensor.matmul(pv2[:], vsb[:, bass.ts(tt, 128)], at[:],
                                     start=(tt == 0), stop=(tt == nt - 1))
                dsb = sts.tile([128, 512], F32, tag="dsb")
                rcb = sts.tile([128, 512], F32, tag="rcb")
                nc.gpsimd.partition_all_reduce(dsb[:], asum[:], channels=128,
                                               reduce_op=RADD)
                nc.vector.reciprocal(rcb[:], dsb[:])
                ah = aop.tile([128, 512], BF, tag="aout")
                nc.vector.tensor_mul(ah[:], pv2[:], rcb[:])
                aout.append(ah)

            # ---- O projection ----
            for dt in range(NDT):
                po = pop.tile([128, 512], F32, tag="o")
                for et in range(NM):
                    nc.tensor.matmul(po[:], wo[:, et, dt], aout[et][:],
                                     start=(et == 0), stop=(et == NM - 1))
                ost = ostp.tile([128, 512], F32, tag="ost")
                nc.scalar.activation(ost[:], po[:], AF.Copy)
                nc.sync.dma_start(out_e[sb, dt], ost[:])


def _prep_shards(x, position_ids, w_q, w_k, w_v, w_o):
    xs = np.ascontiguousarray(x.reshape(S, D_MODEL).T.astype(np.float32))
    x1 = xs.astype(BF16)
    x2 = (xs - x1.astype(np.float32)).astype(BF16)

    def xfmt(a):
        return np.ascontiguousarray(
            a.reshape(NDT, 128, NSB, 512).transpose(2, 0, 1, 3))

    x1s, x2s = xfmt(x1), xfmt(x2)

    pos = position_ids.astype(np.float32)
    inv = 1.0 / (THETA ** (np.arange(0, HD, 2, dtype=np.float32) / HD))  # 64
    ang = pos[None, :] * inv[:, None]          # [64, S]
    cosT = np.empty((128, S), np.float32)      # rows 0:64 cos, 64:128 sin
    cosT[0:64] = np.cos(ang)
    cosT[64:128] = np.sin(ang)

    masks = np.zeros((4, 128, 512), np.float32)  # cast to bf16 below
    jj = np.arange(128)
    cc = np.arange(512)
    for j in range(4):
        masks[j][(128 * j + jj)[:, None] > cc[None, :]] = NEG

    in_maps = []
    for i in range(NC_CORES):
        wq = np.ascontiguousarray(w_q[512 * i:512 * (i + 1)].T)   # [2560,512]
        wq1 = wq.astype(BF16)
        wq2 = (wq - wq1.astype(np.float32)).astype(BF16)

        def qfmt(a):
            # [2560, 512] -> [m, p, dt*128]: A[m, p, dt*128+c] = a[dt*128+p, m*128+c]
            return np.ascontiguousarray(
                a.reshape(NDT, 128, NM, 128).transpose(2, 1, 0, 3).reshape(
                    NM, 128, NDT * 128))

        def kfmt(a):
            # [2560, 128] -> [p, dt*128]: A[p, dt*128+c] = a[dt*128+p, c]
            return np.ascontiguousarray(
                a.reshape(NDT, 128, 128).transpose(1, 0, 2).reshape(128, NDT * 128))

        wk = np.ascontiguousarray(w_k[128 * i:128 * (i + 1)].T)   # [2560,128]
        wk1 = wk.astype(BF16)
        wk2 = (wk - wk1.astype(np.float32)).astype(BF16)
        wv = np.ascontiguousarray(
            w_v[128 * i:128 * (i + 1)].T).astype(BF16)
        wo = np.ascontiguousarray(w_o[:, 512 * i:512 * (i + 1)].T)  # [512,2560]
        # wo tile layout [p=e, (et, dt, c=d)]: A[e, et, dt, d] = wo[et*128+e, dt*128+d]
        wof = np.ascontiguousarray(
            wo.reshape(NM, 128, NDT, 128).transpose(1, 0, 2, 3).reshape(
                128, NM * NDT * 128)).astype(BF16)
        in_maps.append({
            "x1": x1s, "x2": x2s,
            "wq1": qfmt(wq1), "wq2": qfmt(wq2),
            "wk1": kfmt(wk1),
            "wk2": kfmt(wk2),
            "wv": kfmt(wv),
            "wo": wof,
            "cosT": cosT,
            "masks": masks.transpose(1, 0, 2).reshape(128, 4 * 512).astype(BF16),
        })
    return in_maps


def kernel(x, position_ids, w_q, w_k, w_v, w_o):
    x = np.asarray(x); position_ids = np.asarray(position_ids)
    w_q = np.asarray(w_q); w_k = np.asarray(w_k)
    w_v = np.asarray(w_v); w_o = np.asarray(w_o)

    if "nc" not in _cache:
        _cache["nc"] = _build()
    nc = _cache["nc"]

    in_maps = _prep_shards(x, position_ids, w_q, w_k, w_v, w_o)
    res = run_bass_kernel_spmd(nc, in_maps, core_ids=list(range(NC_CORES)))

    outT = np.zeros((NSB, NDT, 128, 512), np.float64)
    ks, vs = [], []
    for i in range(NC_CORES):
        r = res.results[i]
        outT += r["outT"].astype(np.float64)
        ks.append(np.ascontiguousarray(r["kT"].T))                 # [S, 128]
        vs.append(r["vout"].astype(np.float32).reshape(S, HD))
    out = outT.transpose(1, 2, 0, 3).reshape(D_MODEL, S).T.astype(np.float32)
    out = out.reshape(1, S, D_MODEL)
    k = np.stack(ks)[None].astype(np.float32)                      # [1,8,S,128]
    v = np.stack(vs)[None]
    return out, k, v


# revision 38
# speedup vs baseline: 1.0434x; 1.0008x over previous
"""GQA attention (32 Q heads / 8 KV heads, head_dim 128, d_model 2560, s=2048)
with RoPE, tensor-parallel across 8 TRN2 NeuronCores.

Sharding: core i owns Q heads 4i..4i+3 and KV head i (w_q/w_k/w_v sharded on
the head output dim, w_o on its input dim). Each core produces a full-shape
partial of the output projection; the partials are summed on the host (the
"all-reduce after w_o" of the hint, done at unshard time), so no on-device
collective is needed and outputs (k/v caches, out rows) are disjoint/partial.

Precision: the softmax logits here have std ~2.5e3 (softmax is near one-hot),
so Q/K projections and QK^T are computed as 3-pass split-bf16 matmuls
(a1b1+a1b2+a2b1, fp32-equivalent to ~2^-18) while the V chain (V proj,
attn@V, O proj) runs in plain bf16. Measured end-to-end rel err ~3e-3.

Layout: everything runs transposed ([dim, seq] on chip) so that projections,
scores^T, attn@V and the O projection all chain without any transposes.
Softmax runs over the partition axis via gpsimd C-axis reduce + broadcast.
"""
import sys
sys.path.insert(0, '/opt/trn_rl_repo')

import numpy as np
import ml_dtypes

import concourse.bass as bass
import concourse.tile as tile
from concourse import bacc, mybir
from concourse.bass_utils import run_bass_kernel_spmd

BF16 = ml_dtypes.bfloat16
F32 = mybir.dt.float32
BF = mybir.dt.bfloat16

D_MODEL = 2560
NUM_HEADS = 32
NUM_KV = 8
HD = 128
S = 2048
THETA = 5000000.0
NC_CORES = 8
NDT = D_MODEL // 128      # 20 contraction tiles
NSB = S // 512            # 4 sequence blocks of 512
NTT = S // 128            # 16 key tiles of 128
NM = 4                    # q heads per core
SCALE = 1.0 / np.sqrt(HD)
NEG = -1.0e30

_cache = {}


def _build():
    nc = bacc.Bacc("TRN2", target_bir_lowering=False, debug=False,
                   num_devices=NC_CORES)
    dp = nc.declare_dram_parameter
    x1_e = dp("x1", [NSB, NDT, 128, 512], BF, isOutput=False)
    x2_e = dp("x2", [NSB, NDT, 128, 512], BF, isOutput=False)
    wq1_e = dp("wq1", [NM, 128, NDT * 128], BF, isOutput=False)
    wq2_e = dp("wq2", [NM, 128, NDT * 128], BF, isOutput=False)
    wk1_e = dp("wk1", [128, NDT * 128], BF, isOutput=False)
    wk2_e = dp("wk2", [128, NDT * 128], BF, isOutput=False)
    wv_e = dp("wv", [128, NDT * 128], BF, isOutput=False)
    wo_e = dp("wo", [128, NM * NDT * 128], BF, isOutput=False)
    cos_e = dp("cosT", [128, S], F32, isOutput=False)  # rows 0:64 cos, 64:128 sin
    msk_e = dp("masks", [128, 4 * 512], BF, isOutput=False)
    out_e = dp("outT", [NSB, NDT, 128, 512], F32, isOutput=True)
    kt_e = dp("kT", [128, S], F32, isOutput=True)
    v_e = dp("vout", [NTT, 128, 128], BF, isOutput=True)

    with tile.TileContext(nc) as tc:
        _body(nc, tc, x1_e, x2_e, wq1_e, wq2_e, wk1_e, wk2_e, wv_e, wo_e,
              cos_e, msk_e, out_e, kt_e, v_e)
    nc.compile()
    return nc


def _body(nc, tc, x1_e, x2_e, wq1_e, wq2_e, wk1_e, wk2_e, wv_e, wo_e,
          cos_e, msk_e, out_e, kt_e, v_e):
    AF = mybir.ActivationFunctionType
    OP = mybir.AluOpType
    from concourse import bass_isa
    from concourse.masks import make_identity
    RMAX = bass_isa.ReduceOp.max
    RADD = bass_isa.ReduceOp.add
    with (
        tc.tile_pool(name="wres", bufs=1) as wres,      # resident weights/tables
        tc.tile_pool(name="kv", bufs=1) as kvp,         # k1,k2,v resident
        tc.tile_pool(name="xp", bufs=NDT) as xp,        # x1 resident / x2 stream
        tc.tile_pool(name="stg", bufs=1) as stg,      # fp32 score staging
        tc.tile_pool(name="sml", bufs=2) as sml,        # small rotating scratch
        tc.tile_pool(name="att", bufs=2) as att,        # bf16 attn weights
        tc.tile_pool(name="sts", bufs=1) as sts,        # softmax stats
        tc.tile_pool(name="ao", bufs=4) as aop,         # attn out per head
        tc.tile_pool(name="ost", bufs=2) as ostp,       # out staging
        tc.tile_pool(name="pacc", bufs=3, space="PSUM") as pacc,
        tc.tile_pool(name="psc", bufs=2, space="PSUM") as pscp,
        tc.tile_pool(name="pav", bufs=1, space="PSUM") as pavp,
        tc.tile_pool(name="po", bufs=2, space="PSUM") as pop,
    ):
        # resident loads (partition-major dram layouts; all contiguous DMAs)
        wk1 = wres.tile([128, NDT, 128], BF)
        wk2 = wres.tile([128, NDT, 128], BF)
        wv = wres.tile([128, NDT, 128], BF)
        nc.scalar.dma_start(wk1[:], wk1_e[:].rearrange("p (a b) -> p a b", b=128))
        nc.scalar.dma_start(wk2[:], wk2_e[:].rearrange("p (a b) -> p a b", b=128))
        nc.scalar.dma_start(wv[:], wv_e[:].rearrange("p (a b) -> p a b", b=128))
        wq1m, wq2m = [], []
        for m in range(NM):
            w1 = wres.tile([128, NDT, 128], BF, name=f"wq1m{m}")
            w2 = wres.tile([128, NDT, 128], BF, name=f"wq2m{m}")
            nc.scalar.dma_start(w1[:], wq1_e[m].rearrange("p (a b) -> p a b", b=128))
            nc.scalar.dma_start(w2[:], wq2_e[m].rearrange("p (a b) -> p a b", b=128))
            wq1m.append(w1)
            wq2m.append(w2)
        tbl = wres.tile([128, S], F32)   # rows 0:64 cos, rows 64:128 sin
        masks = wres.tile([128, 4, 512], BF)
        ident = wres.tile([128, 128], BF)
        make_identity(nc, ident)
        nc.scalar.dma_start(tbl[:], cos_e[:])
        nc.scalar.dma_start(masks[:], msk_e[:].rearrange("p (a b) -> p a b", b=512))
        wo = wres.tile([128, NM, NDT, 128], BF)
        nc.scalar.dma_start(wo[:], wo_e[:].rearrange("p (a b c) -> p a b c",
                                                   b=NDT, c=128))

        stgA = stg.tile([128, 8 * 512], F32)   # score staging, key tiles 0..7
        stgB = stg.tile([128, 8 * 512], F32)   # score staging, key tiles 8..15
        k1 = kvp.tile([128, S], BF)
        k2 = kvp.tile([128, S], BF)
        vsb = kvp.tile([128, S], BF)   # [t_local, tt*128+e]

        def rope(psrc, cs):
            """rope a [128,512] psum tile -> fp32 sbuf tile (5 DVE ops)"""
            t1 = sml.tile([128, 512], F32, tag="t1")
            t2 = sml.tile([128, 512], F32, tag="t2")
            r = sml.tile([128, 512], F32, tag="rope")
            nc.vector.tensor_mul(t1[0:64, :], psrc[0:64, :], tbl[0:64, cs])
            nc.vector.tensor_mul(t1[64:128, :], psrc[64:128, :], tbl[0:64, cs])
            nc.vector.tensor_mul(t2[0:64, :], psrc[64:128, :], tbl[64:128, cs])
            nc.vector.tensor_mul(t2[64:128, :], psrc[0:64, :], tbl[64:128, cs])
            nc.vector.tensor_sub(r[0:64, :], t1[0:64, :], t2[0:64, :])
            nc.vector.tensor_add(r[64:128, :], t1[64:128, :], t2[64:128, :])
            return r

        for sb in range(NSB):
            cs = bass.ts(sb, 512)       # column slice for this seq block
            nt = 4 * (sb + 1)           # causal key tiles

            x1t = []
            for dt in range(NDT):
                a = xp.tile([128, 512], BF, tag="x1")
                nc.sync.dma_start(a[:], x1_e[sb, dt])
                x1t.append(a)

            def x2tile(dt):
                b = xp.tile([128, 512], BF, tag="x2", bufs=6)
                nc.sync.dma_start(b[:], x2_e[sb, dt])
                return b

            # ---- K (x1 passes) + vT in one dt pass; K x2 pass rides pair 0 ----
            pk = pacc.tile([128, 512], F32, tag="acc")
            pvT = pacc.tile([128, 512], F32, tag="acc")
            for dt in range(NDT):
                fst, lst = dt == 0, dt == NDT - 1
                nc.tensor.matmul(pk[:], wk1[:, dt], x1t[dt][:],
                                 start=fst, stop=False)
                nc.tensor.matmul(pk[:], wk2[:, dt], x1t[dt][:],
                                 start=False, stop=False)
                nc.tensor.matmul(pvT[:], wv[:, dt], x1t[dt][:],
                                 start=fst, stop=lst)

            vT_sb = sml.tile([128, 512], BF, tag="vts")
            nc.scalar.activation(vT_sb[:], pvT[:], AF.Copy)
            for tt in range(4):
                g = 4 * sb + tt
                tp = pscp.tile([128, 128], BF, tag="sc", name=f"tp{g}")
                nc.tensor.transpose(tp[:], vT_sb[:, bass.ts(tt, 128)], ident[:])
                nc.scalar.activation(vsb[:, bass.ts(g, 128)], tp[:], AF.Copy)
                nc.sync.dma_start(v_e[g], vsb[:, bass.ts(g, 128)])

            # ---- Q projection in head pairs + rope + scaled split ----
            q1h, q2h = [None] * NM, [None] * NM
            for pair in range(2):
                mA, mB = 2 * pair, 2 * pair + 1
                pqA = pacc.tile([128, 512], F32, tag="acc", name=f"pqA{pair}")
                pqB = pacc.tile([128, 512], F32, tag="acc", name=f"pqB{pair}")
                for dt in range(NDT):
                    fst = dt == 0
                    x2 = x2tile(dt)
                    if pair == 0:
                        nc.tensor.matmul(pk[:], wk1[:, dt], x2[:],
                                         start=False, stop=(dt == NDT - 1))
                    nc.tensor.matmul(pqA[:], wq1m[mA][:, dt], x2[:],
                                     start=fst, stop=False)
                    nc.tensor.matmul(pqB[:], wq1m[mB][:, dt], x2[:],
                                     start=fst, stop=False)
                if pair == 0:
                    kTs = rope(pk, cs)
                    nc.scalar.activation(k1[:, cs], kTs[:], AF.Copy)
                    k1f = sml.tile([128, 512], F32, tag="sub")
                    nc.scalar.activation(k1f[:], k1[:, cs], AF.Copy)
                    nc.vector.tensor_sub(k2[:, cs], kTs[:], k1f[:])
                    nc.sync.dma_start(kt_e[:, cs], kTs[:])
                for dt in range(NDT):
                    lst = dt == NDT - 1
                    nc.tensor.matmul(pqA[:], wq1m[mA][:, dt], x1t[dt][:],
                                     start=False, stop=False)
                    nc.tensor.matmul(pqA[:], wq2m[mA][:, dt], x1t[dt][:],
                                     start=False, stop=lst)
                for dt in range(NDT):
                    lst = dt == NDT - 1
                    nc.tensor.matmul(pqB[:], wq1m[mB][:, dt], x1t[dt][:],
                                     start=False, stop=False)
                    nc.tensor.matmul(pqB[:], wq2m[mB][:, dt], x1t[dt][:],
                                     start=False, stop=lst)
                for m, pq in ((mA, pqA), (mB, pqB)):
                    qTs = rope(pq, cs)
                    q1 = aop.tile([128, 512], BF, tag="q1")
                    q2 = aop.tile([128, 512], BF, tag="q2")
                    nc.scalar.activation(q1[:], qTs[:], AF.Copy, scale=SCALE)
                    q1f = sml.tile([128, 512], F32, tag="sub")
                    nc.scalar.activation(q1f[:], q1[:], AF.Copy)
                    nc.vector.scalar_tensor_tensor(q2[:], qTs[:], SCALE, q1f[:],
                                                   op0=OP.mult, op1=OP.subtract)
                    q1h[m], q2h[m] = q1, q2

            # ---- attention per head ----
            aout = []
            for h in range(NM):
                amax = sts.tile([128, 512], F32, tag="amax")
                for tt in range(nt):
                    ks = bass.ts(tt, 128)
                    ps = pscp.tile([128, 512], F32, tag="sc")
                    nc.tensor.matmul(ps[:], k1[:, ks], q1h[h][:],
                                     start=True, stop=False)
                    nc.tensor.matmul(ps[:], k1[:, ks], q2h[h][:],
                                     start=False, stop=False)
                    nc.tensor.matmul(ps[:], k2[:, ks], q1h[h][:],
                                     start=False, stop=True)
                    st = (stgA if tt < 8 else stgB)[:, bass.ts(tt % 8, 512)]
                    j = tt - 4 * sb
                    if j >= 0:
                        nc.vector.tensor_add(st, ps[:], masks[:, j])
                    else:
                        nc.scalar.activation(st, ps[:], AF.Copy)
                # strided reduce per quarter (4 tiles) so the chain tail is short
                na = min(nt, 8)
                nq = nt // 4
                amaxq = sts.tile([128, 512], F32, tag="amaxB")
                for q in range(nq):
                    half = stgA if q < 2 else stgB
                    off = (q % 2) * 4 * 512
                    sv = half[:, off:off + 4 * 512].rearrange(
                        "p (t c) -> p c t", c=512)
                    if q == 0:
                        nc.vector.tensor_reduce(
                            amax[:], sv, axis=mybir.AxisListType.X, op=OP.max)
                    else:
                        nc.vector.tensor_reduce(
                            amaxq[:], sv, axis=mybir.AxisListType.X, op=OP.max)
                        nc.vector.tensor_max(amax[:], amax[:], amaxq[:])
                mxb = sts.tile([128, 512], F32, tag="mxb")
                nc.gpsimd.partition_all_reduce(mxb[:], amax[:], channels=128,
                                               reduce_op=RMAX)
                pv2 = pavp.tile([128, 512], F32, tag="av")
                asum = sts.tile([128, 512], F32, tag="asum")
                # first tile subtracted alone so exp/attnV can start immediately
                nc.vector.tensor_sub(stgA[:, 0:512], stgA[:, 0:512], mxb[:])
                blkA = stgA[:, 512:na * 512].rearrange("p (t c) -> p t c", c=512)
                nc.vector.tensor_sub(
                    blkA, blkA,
                    mxb[:].unsqueeze(1).broadcast_to([128, na - 1, 512]))
                if nt > 8:
                    blkB = stgB[:, 0:(nt - 8) * 512].rearrange(
                        "p (t c) -> p t c", c=512)
                    nc.vector.tensor_sub(
                        blkB, blkB,
                        mxb[:].unsqueeze(1).broadcast_to([128, nt - 8, 512]))
                for tt in range(nt):
                    sl = (stgA if tt < 8 else stgB)[:, bass.ts(tt % 8, 512)]
                    at = att.tile([128, 512], BF, tag="at")
                    nc.scalar.activation(at[:], sl, AF.Exp)
                    if tt == 0:
                        nc.vector.tensor_copy(asum[:], at[:])
                    else:
                        nc.vector.tensor_add(asum[:], asum[:], at[:])
                    nc.tensor.matmul(pv2[:], vsb[:, bass.ts(tt, 128)], at[:],
                                     start=(tt == 0), stop=(tt == nt - 1))
                dsb = sts.tile([128, 512], F32, tag="dsb")
                rcb = sts.tile([128, 512], F32, tag="rcb")
                nc.gpsimd.partition_all_reduce(dsb[:], asum[:], channels=128,
                                               reduce_op=RADD)
                nc.vector.reciprocal(rcb[:], dsb[:])
                ah = aop.tile([128, 512], BF, tag="aout")
                nc.vector.tensor_mul(ah[:], pv2[:], rcb[:])
                aout.append(ah)

            # ---- O projection ----
            for dt in range(NDT):
                po = pop.tile([128, 512], F32, tag="o")
                for et in range(NM):
                    nc.tensor.matmul(po[:], wo[:, et, dt], aout[et][:],
                                     start=(et == 0), stop=(et == NM - 1))
                ost = ostp.tile([128, 512], F32, tag="ost")
                nc.scalar.activation(ost[:], po[:], AF.Copy)
                nc.sync.dma_start(out_e[sb, dt], ost[:])


def _prep_shards(x, position_ids, w_q, w_k, w_v, w_o):
    xs = np.ascontiguousarray(x.reshape(S, D_MODEL).T.astype(np.float32))
    x1 = xs.astype(BF16)
    x2 = (xs - x1.astype(np.float32)).astype(BF16)

    def xfmt(a):
        return np.ascontiguousarray(
            a.reshape(NDT, 128, NSB, 512).transpose(2, 0, 1, 3))

    x1s, x2s = xfmt(x1), xfmt(x2)

    pos = position_ids.astype(np.float32)
    inv = 1.0 / (THETA ** (np.arange(0, HD, 2, dtype=np.float32) / HD))  # 64
    ang = pos[None, :] * inv[:, None]          # [64, S]
    cosT = np.empty((128, S), np.float32)      # rows 0:64 cos, 64:128 sin
    cosT[0:64] = np.cos(ang)
    cosT[64:128] = np.sin(ang)

    masks = np.zeros((4, 128, 512), np.float32)  # cast to bf16 below
    jj = np.arange(128)
    cc = np.arange(512)
    for j in range(4):
        masks[j][(128 * j + jj)[:, None] > cc[None, :]] = NEG

    in_maps = []
    for i in range(NC_CORES):
        wq = np.ascontiguousarray(w_q[512 * i:512 * (i + 1)].T)   # [2560,512]
        wq1 = wq.astype(BF16)
        wq2 = (wq - wq1.astype(np.float32)).astype(BF16)

        def qfmt(a):
            # [2560, 512] -> [m, p, dt*128]: A[m, p, dt*128+c] = a[dt*128+p, m*128+c]
            return np.ascontiguousarray(
                a.reshape(NDT, 128, NM, 128).transpose(2, 1, 0, 3).reshape(
                    NM, 128, NDT * 128))

        def kfmt(a):
            # [2560, 128] -> [p, dt*128]: A[p, dt*128+c] = a[dt*128+p, c]
            return np.ascontiguousarray(
                a.reshape(NDT, 128, 128).transpose(1, 0, 2).reshape(128, NDT * 128))

        wk = np.ascontiguousarray(w_k[128 * i:128 * (i + 1)].T)   # [2560,128]
        wk1 = wk.astype(BF16)
        wk2 = (wk - wk1.astype(np.float32)).astype(BF16)
        wv = np.ascontiguousarray(
            w_v[128 * i:128 * (i + 1)].T).astype(BF16)
        wo = np.ascontiguousarray(w_o[:, 512 * i:512 * (i + 1)].T)  # [512,2560]
        # wo tile layout [p=e, (et, dt, c=d)]: A[e, et, dt, d] = wo[et*128+e, dt*128+d]
        wof = np.ascontiguousarray(
            wo.reshape(NM, 128, NDT, 128).transpose(1, 0, 2, 3).reshape(
                128, NM * NDT * 128)).astype(BF16)
        in_maps.append({
            "x1": x1s, "x2": x2s,
            "wq1": qfmt(wq1), "wq2": qfmt(wq2),
            "wk1": kfmt(wk1),
            "wk2": kfmt(wk2),
            "wv": kfmt(wv),
            "wo": wof,
            "cosT": cosT,
            "masks": masks.transpose(1, 0, 2).reshape(128, 4 * 512).astype(BF16),
        })
    return in_maps


def kernel(x, position_ids, w_q, w_k, w_v, w_o):
    x = np.asarray(x); position_ids = np.asarray(position_ids)
    w_q = np.asarray(w_q); w_k = np.asarray(w_k)
    w_v = np.asarray(w_v); w_o = np.asarray(w_o)

    if "nc" not in _cache:
        _cache["nc"] = _build()
    nc = _cache["nc"]

    in_maps = _prep_shards(x, position_ids, w_q, w_k, w_v, w_o)
    res = run_bass_kernel_spmd(nc, in_maps, core_ids=list(range(NC_CORES)))

    outT = np.zeros((NSB, NDT, 128, 512), np.float64)
    ks, vs = [], []
    for i in range(NC_CORES):
        r = res.results[i]
        outT += r["outT"].astype(np.float64)
        ks.append(np.ascontiguousarray(r["kT"].T))                 # [S, 128]
        vs.append(r["vout"].astype(np.float32).reshape(S, HD))
    out = outT.transpose(1, 2, 0, 3).reshape(D_MODEL, S).T.astype(np.float32)
    out = out.reshape(1, S, D_MODEL)
    k = np.stack(ks)[None].astype(np.float32)                      # [1,8,S,128]
    v = np.stack(vs)[None]
    return out, k, v


# revision 43
# speedup vs baseline: 1.0506x; 1.0069x over previous
"""GQA attention (32 Q heads / 8 KV heads, head_dim 128, d_model 2560, s=2048)
with RoPE, tensor-parallel across 8 TRN2 NeuronCores.

Sharding: core i owns Q heads 4i..4i+3 and KV head i (w_q/w_k/w_v sharded on
the head output dim, w_o on its input dim). Each core produces a full-shape
partial of the output projection; the partials are summed on the host (the
"all-reduce after w_o" of the hint, done at unshard time), so no on-device
collective is needed and outputs (k/v caches, out rows) are disjoint/partial.

Precision: the softmax logits here have std ~2.5e3 (softmax is near one-hot),
so Q/K projections and QK^T are computed as 3-pass split-bf16 matmuls
(a1b1+a1b2+a2b1, fp32-equivalent to ~2^-18) while the V chain (V proj,
attn@V, O proj) runs in plain bf16. Measured end-to-end rel err ~3e-3.

Layout: everything runs transposed ([dim, seq] on chip) so that projections,
scores^T, attn@V and the O projection all chain without any transposes.
Softmax runs over the partition axis via gpsimd C-axis reduce + broadcast.
"""
import sys
sys.path.insert(0, '/opt/trn_rl_repo')

import numpy as np
import ml_dtypes

import concourse.bass as bass
import concourse.tile as tile
from concourse import bacc, mybir
from concourse.bass_utils import run_bass_kernel_spmd

BF16 = ml_dtypes.bfloat16
F32 = mybir.dt.float32
BF = mybir.dt.bfloat16

D_MODEL = 2560
NUM_HEADS = 32
NUM_KV = 8
HD = 128
S = 2048
THETA = 5000000.0
NC_CORES = 8
NDT = D_MODEL // 128      # 20 contraction tiles
NSB = S // 512            # 4 sequence blocks of 512
NTT = S // 128            # 16 key tiles of 128
NM = 4                    # q heads per core
SCALE = 1.0 / np.sqrt(HD)
NEG = -1.0e30

_cache = {}


def _build():
    nc = bacc.Bacc("TRN2", target_bir_lowering=False, debug=False,
                   num_devices=NC_CORES)
    dp = nc.declare_dram_parameter
    x1_e = dp("x1", [NSB, NDT, 128, 512], BF, isOutput=False)
    x2_e = dp("x2", [NSB, NDT, 128, 512], BF, isOutput=False)
    wq1_e = dp("wq1", [NM, 128, NDT * 128], BF, isOutput=False)
    wq2_e = dp("wq2", [NM, 128, NDT * 128], BF, isOutput=False)
    wk1_e = dp("wk1", [128, NDT * 128], BF, isOutput=False)
    wk2_e = dp("wk2", [128, NDT * 128], BF, isOutput=False)
    wv_e = dp("wv", [128, NDT * 128], BF, isOutput=False)
    wo_e = dp("wo", [128, NM * NDT * 128], BF, isOutput=False)
    cos_e = dp("cosT", [128, S], F32, isOutput=False)  # rows 0:64 cos, 64:128 sin
    msk_e = dp("masks", [128, 4 * 512], BF, isOutput=False)
    out_e = dp("outT", [NSB, NDT, 128, 512], F32, isOutput=True)
    kt_e = dp("kT", [128, S], F32, isOutput=True)
    v_e = dp("vout", [NTT, 128, 128], BF, isOutput=True)

    with tile.TileContext(nc) as tc:
        _body(nc, tc, x1_e, x2_e, wq1_e, wq2_e, wk1_e, wk2_e, wv_e, wo_e,
              cos_e, msk_e, out_e, kt_e, v_e)
    nc.compile()
    return nc


def _body(nc, tc, x1_e, x2_e, wq1_e, wq2_e, wk1_e, wk2_e, wv_e, wo_e,
          cos_e, msk_e, out_e, kt_e, v_e):
    AF = mybir.ActivationFunctionType
    OP = mybir.AluOpType
    from concourse import bass_isa
    from concourse.masks import make_identity
    RMAX = bass_isa.ReduceOp.max
    RADD = bass_isa.ReduceOp.add
    with (
        tc.tile_pool(name="wres", bufs=1) as wres,      # resident weights/tables
        tc.tile_pool(name="kv", bufs=1) as kvp,         # k1,k2,v resident
        tc.tile_pool(name="xp", bufs=NDT) as xp,        # x1 resident / x2 stream
        tc.tile_pool(name="stg", bufs=1) as stg,      # fp32 score staging
        tc.tile_pool(name="sml", bufs=2) as sml,        # small rotating scratch
        tc.tile_pool(name="att", bufs=2) as att,        # bf16 attn weights
        tc.tile_pool(name="sts", bufs=1) as sts,        # softmax stats
        tc.tile_pool(name="ao", bufs=4) as aop,         # attn out per head
        tc.tile_pool(name="ost", bufs=2) as ostp,       # out staging
        tc.tile_pool(name="pacc", bufs=3, space="PSUM") as pacc,
        tc.tile_pool(name="psc", bufs=2, space="PSUM") as pscp,
        tc.tile_pool(name="pav", bufs=1, space="PSUM") as pavp,
        tc.tile_pool(name="po", bufs=2, space="PSUM") as pop,
    ):
        # resident loads (partition-major dram layouts; all contiguous DMAs)
        wk1 = wres.tile([128, NDT, 128], BF)
        wk2 = wres.tile([128, NDT, 128], BF)
        wv = wres.tile([128, NDT, 128], BF)
        nc.scalar.dma_start(wk1[:], wk1_e[:].rearrange("p (a b) -> p a b", b=128))
        nc.scalar.dma_start(wk2[:], wk2_e[:].rearrange("p (a b) -> p a b", b=128))
        nc.scalar.dma_start(wv[:], wv_e[:].rearrange("p (a b) -> p a b", b=128))
        wq1m, wq2m = [], []
        for m in range(NM):
            w1 = wres.tile([128, NDT, 128], BF, name=f"wq1m{m}")
            w2 = wres.tile([128, NDT, 128], BF, name=f"wq2m{m}")
            nc.scalar.dma_start(w1[:], wq1_e[m].rearrange("p (a b) -> p a b", b=128))
            nc.scalar.dma_start(w2[:], wq2_e[m].rearrange("p (a b) -> p a b", b=128))
            wq1m.append(w1)
            wq2m.append(w2)
        tbl = wres.tile([128, S], F32)   # rows 0:64 cos, rows 64:128 sin
        masks = wres.tile([128, 4, 512], BF)
        ident = wres.tile([128, 128], BF)
        make_identity(nc, ident)
        nc.scalar.dma_start(tbl[:], cos_e[:])
        nc.scalar.dma_start(masks[:], msk_e[:].rearrange("p (a b) -> p a b", b=512))
        wo = wres.tile([128, NM, NDT, 128], BF)
        nc.scalar.dma_start(wo[:], wo_e[:].rearrange("p (a b c) -> p a b c",
                                                   b=NDT, c=128))

        stgA = stg.tile([128, 8 * 512], F32)   # score staging, key tiles 0..7
        stgB = stg.tile([128, 8 * 512], F32)   # score staging, key tiles 8..15
        k1 = kvp.tile([128, S], BF)
        k2 = kvp.tile([128, S], BF)
        vsb = kvp.tile([128, S], BF)   # [t_local, tt*128+e]

        def rope(psrc, cs):
            """rope a [128,512] psum tile -> fp32 sbuf tile (5 DVE ops)"""
            t1 = sml.tile([128, 512], F32, tag="t1")
            t2 = sml.tile([128, 512], F32, tag="t2")
            r = sml.tile([128, 512], F32, tag="rope")
            nc.vector.tensor_mul(t1[0:64, :], psrc[0:64, :], tbl[0:64, cs])
            nc.vector.tensor_mul(t1[64:128, :], psrc[64:128, :], tbl[0:64, cs])
            nc.vector.tensor_mul(t2[0:64, :], psrc[64:128, :], tbl[64:128, cs])
            nc.vector.tensor_mul(t2[64:128, :], psrc[0:64, :], tbl[64:128, cs])
            nc.vector.tensor_sub(r[0:64, :], t1[0:64, :], t2[0:64, :])
            nc.vector.tensor_add(r[64:128, :], t1[64:128, :], t2[64:128, :])
            return r

        for sb in range(NSB):
            cs = bass.ts(sb, 512)       # column slice for this seq block
            nt = 4 * (sb + 1)           # causal key tiles

            x1t, x2t0 = [], []
            for dt in range(NDT):
                a = xp.tile([128, 512], BF, tag="x1")
                nc.sync.dma_start(a[:], x1_e[sb, dt])
                x1t.append(a)
                b = xp.tile([128, 512], BF, tag="x2", bufs=8, name=f"x2p{dt}")
                nc.sync.dma_start(b[:], x2_e[sb, dt])
                x2t0.append(b)

            def x2tile(dt):
                b = xp.tile([128, 512], BF, tag="x2", bufs=8)
                nc.sync.dma_start(b[:], x2_e[sb, dt])
                return b

            # ---- K (x1 passes) + vT in one dt pass; K x2 pass rides pair 0 ----
            pk = pacc.tile([128, 512], F32, tag="acc")
            pvT = pacc.tile([128, 512], F32, tag="acc")
            for dt in range(NDT):
                fst, lst = dt == 0, dt == NDT - 1
                nc.tensor.matmul(pk[:], wk1[:, dt], x1t[dt][:],
                                 start=fst, stop=False)
                nc.tensor.matmul(pk[:], wk2[:, dt], x1t[dt][:],
                                 start=False, stop=False)
                nc.tensor.matmul(pvT[:], wv[:, dt], x1t[dt][:],
                                 start=fst, stop=lst)

            vT_sb = sml.tile([128, 512], BF, tag="vts")
            nc.scalar.activation(vT_sb[:], pvT[:], AF.Copy)
            for tt in range(4):
                g = 4 * sb + tt
                tp = pscp.tile([128, 128], BF, tag="sc", name=f"tp{g}")
                nc.tensor.transpose(tp[:], vT_sb[:, bass.ts(tt, 128)], ident[:])
                nc.scalar.activation(vsb[:, bass.ts(g, 128)], tp[:], AF.Copy)
                nc.sync.dma_start(v_e[g], vsb[:, bass.ts(g, 128)])

            # ---- Q projection in head pairs + rope + scaled split ----
            q1h, q2h = [None] * NM, [None] * NM
            for pair in range(2):
                mA, mB = 2 * pair, 2 * pair + 1
                pqA = pacc.tile([128, 512], F32, tag="acc", name=f"pqA{pair}")
                pqB = pacc.tile([128, 512], F32, tag="acc", name=f"pqB{pair}")
                for dt in range(NDT):
                    fst = dt == 0
                    x2 = x2t0[dt] if pair == 0 else x2tile(dt)
                    if pair == 0:
                        nc.tensor.matmul(pk[:], wk1[:, dt], x2[:],
                                         start=False, stop=(dt == NDT - 1))
                    nc.tensor.matmul(pqA[:], wq1m[mA][:, dt], x2[:],
                                     start=fst, stop=False)
                    nc.tensor.matmul(pqB[:], wq1m[mB][:, dt], x2[:],
                                     start=fst, stop=False)
                if pair == 0:
                    kTs = rope(pk, cs)
                    nc.scalar.activation(k1[:, cs], kTs[:], AF.Copy)
                    k1f = sml.tile([128, 512], F32, tag="sub")
                    nc.scalar.activation(k1f[:], k1[:, cs], AF.Copy)
                    nc.vector.tensor_sub(k2[:, cs], kTs[:], k1f[:])
                    nc.sync.dma_start(kt_e[:, cs], kTs[:])
                for dt in range(NDT):
                    lst = dt == NDT - 1
                    nc.tensor.matmul(pqA[:], wq1m[mA][:, dt], x1t[dt][:],
                                     start=False, stop=False)
                    nc.tensor.matmul(pqA[:], wq2m[mA][:, dt], x1t[dt][:],
                                     start=False, stop=lst)
                for dt in range(NDT):
                    lst = dt == NDT - 1
                    nc.tensor.matmul(pqB[:], wq1m[mB][:, dt], x1t[dt][:],
                                     start=False, stop=False)
                    nc.tensor.matmul(pqB[:], wq2m[mB][:, dt], x1t[dt][:],
                                     start=False, stop=lst)
                for m, pq in ((mA, pqA), (mB, pqB)):
                    qTs = rope(pq, cs)
                    q1 = aop.tile([128, 512], BF, tag="q1")
                    q2 = aop.tile([128, 512], BF, tag="q2")
                    nc.scalar.activation(q1[:], qTs[:], AF.Copy, scale=SCALE)
                    q1f = sml.tile([128, 512], F32, tag="sub")
                    nc.scalar.activation(q1f[:], q1[:], AF.Copy)
                    nc.vector.scalar_tensor_tensor(q2[:], qTs[:], SCALE, q1f[:],
                                                   op0=OP.mult, op1=OP.subtract)
                    q1h[m], q2h[m] = q1, q2

            # ---- attention per head ----
            aout = []
            for h in range(NM):
                amax = sts.tile([128, 512], F32, tag="amax")
                for tt in range(nt):
                    ks = bass.ts(tt, 128)
                    ps = pscp.tile([128, 512], F32, tag="sc")
                    nc.tensor.matmul(ps[:], k1[:, ks], q1h[h][:],
                                     start=True, stop=False)
                    nc.tensor.matmul(ps[:], k1[:, ks], q2h[h][:],
                                     start=False, stop=False)
                    nc.tensor.matmul(ps[:], k2[:, ks], q1h[h][:],
                                     start=False, stop=True)
                    st = (stgA if tt < 8 else stgB)[:, bass.ts(tt % 8, 512)]
                    j = tt - 4 * sb
                    if j >= 0:
                        nc.vector.tensor_add(st, ps[:], masks[:, j])
                    else:
                        nc.scalar.activation(st, ps[:], AF.Copy)
                # strided reduce per quarter (4 tiles) so the chain tail is short
                na = min(nt, 8)
                nq = nt // 4
                amaxq = sts.tile([128, 512], F32, tag="amaxB")
                for q in range(nq):
                    half = stgA if q < 2 else stgB
                    off = (q % 2) * 4 * 512
                    sv = half[:, off:off + 4 * 512].rearrange(
                        "p (t c) -> p c t", c=512)
                    if q == 0:
                        nc.vector.tensor_reduce(
                            amax[:], sv, axis=mybir.AxisListType.X, op=OP.max)
                    else:
                        nc.vector.tensor_reduce(
                            amaxq[:], sv, axis=mybir.AxisListType.X, op=OP.max)
                        nc.vector.tensor_max(amax[:], amax[:], amaxq[:])
                mxb = sts.tile([128, 512], F32, tag="mxb")
                nc.gpsimd.partition_all_reduce(mxb[:], amax[:], channels=128,
                                               reduce_op=RMAX)
                pv2 = pavp.tile([128, 512], F32, tag="av")
                asum = sts.tile([128, 512], F32, tag="asum")
                # first tile subtracted alone so exp/attnV can start immediately
                nc.vector.tensor_sub(stgA[:, 0:512], stgA[:, 0:512], mxb[:])
                blkA = stgA[:, 512:na * 512].rearrange("p (t c) -> p t c", c=512)
                nc.vector.tensor_sub(
                    blkA, blkA,
                    mxb[:].unsqueeze(1).broadcast_to([128, na - 1, 512]))
                if nt > 8:
                    blkB = stgB[:, 0:(nt - 8) * 512].rearrange(
                        "p (t c) -> p t c", c=512)
                    nc.vector.tensor_sub(
                        blkB, blkB,
                        mxb[:].unsqueeze(1).broadcast_to([128, nt - 8, 512]))
                for tt in range(nt):
                    sl = (stgA if tt < 8 else stgB)[:, bass.ts(tt % 8, 512)]
                    at = att.tile([128, 512], BF, tag="at")
                    nc.scalar.activation(at[:], sl, AF.Exp)
                    if tt == 0:
                        nc.vector.tensor_copy(asum[:], at[:])
                    else:
                        nc.vector.tensor_add(asum[:], asum[:], at[:])
                    nc.tensor.matmul(pv2[:], vsb[:, bass.ts(tt, 128)], at[:],
                                     start=(tt == 0), stop=(tt == nt - 1))
                dsb = sts.tile([128, 512], F32, tag="dsb")
                rcb = sts.tile([128, 512], F32, tag="rcb")
                nc.gpsimd.partition_all_reduce(dsb[:], asum[:], channels=128,
                                               reduce_op=RADD)
                nc.vector.reciprocal(rcb[:], dsb[:])
                ah = aop.tile([128, 512], BF, tag="aout")
                nc.vector.tensor_mul(ah[:], pv2[:], rcb[:])
                aout.append(ah)

            # ---- O projection ----
            for dt in range(NDT):
                po = pop.tile([128, 512], F32, tag="o")
                for et in range(NM):
                    nc.tensor.matmul(po[:], wo[:, et, dt], aout[et][:],
                                     start=(et == 0), stop=(et == NM - 1))
                ost = ostp.tile([128, 512], F32, tag="ost")
                nc.scalar.activation(ost[:], po[:], AF.Copy)
                nc.sync.dma_start(out_e[sb, dt], ost[:])


def _prep_shards(x, position_ids, w_q, w_k, w_v, w_o):
    xs = np.ascontiguousarray(x.reshape(S, D_MODEL).T.astype(np.float32))
    x1 = xs.astype(BF16)
    x2 = (xs - x1.astype(np.float32)).astype(BF16)

    def xfmt(a):
        return np.ascontiguousarray(
            a.reshape(NDT, 128, NSB, 512).transpose(2, 0, 1, 3))

    x1s, x2s = xfmt(x1), xfmt(x2)

    pos = position_ids.astype(np.float32)
    inv = 1.0 / (THETA ** (np.arange(0, HD, 2, dtype=np.float32) / HD))  # 64
    ang = pos[None, :] * inv[:, None]          # [64, S]
    cosT = np.empty((128, S), np.float32)      # rows 0:64 cos, 64:128 sin
    cosT[0:64] = np.cos(ang)
    cosT[64:128] = np.sin(ang)

    masks = np.zeros((4, 128, 512), np.float32)  # cast to bf16 below
    jj = np.arange(128)
    cc = np.arange(512)
    for j in range(4):
        masks[j][(128 * j + jj)[:, None] > cc[None, :]] = NEG

    in_maps = []
    for i in range(NC_CORES):
        wq = np.ascontiguousarray(w_q[512 * i:512 * (i + 1)].T)   # [2560,512]
        wq1 = wq.astype(BF16)
        wq2 = (wq - wq1.astype(np.float32)).astype(BF16)

        def qfmt(a):
            # [2560, 512] -> [m, p, dt*128]: A[m, p, dt*128+c] = a[dt*128+p, m*128+c]
            return np.ascontiguousarray(
                a.reshape(NDT, 128, NM, 128).transpose(2, 1, 0, 3).reshape(
                    NM, 128, NDT * 128))

        def kfmt(a):
            # [2560, 128] -> [p, dt*128]: A[p, dt*128+c] = a[dt*128+p, c]
            return np.ascontiguousarray(
                a.reshape(NDT, 128, 128).transpose(1, 0, 2).reshape(128, NDT * 128))

        wk = np.ascontiguousarray(w_k[128 * i:128 * (i + 1)].T)   # [2560,128]
        wk1 = wk.astype(BF16)
        wk2 = (wk - wk1.astype(np.float32)).astype(BF16)
        wv = np.ascontiguousarray(
            w_v[128 * i:128 * (i + 1)].T).astype(BF16)
        wo = np.ascontiguousarray(w_o[:, 512 * i:512 * (i + 1)].T)  # [512,2560]
        # wo tile layout [p=e, (et, dt, c=d)]: A[e, et, dt, d] = wo[et*128+e, dt*128+d]
        wof = np.ascontiguousarray(
            wo.reshape(NM, 128, NDT, 128).transpose(1, 0, 2, 3).reshape(
                128, NM * NDT * 128)).astype(BF16)
        in_maps.append({
            "x1": x1s, "x2": x2s,
            "wq1": qfmt(wq1), "wq2": qfmt(wq2),
            "wk1": kfmt(wk1),
            "wk2": kfmt(wk2),
            "wv": kfmt(wv),
            "wo": wof,
            "cosT": cosT,
            "masks": masks.transpose(1, 0, 2).reshape(128, 4 * 512).astype(BF16),
        })
    return in_maps


def kernel(x, position_ids, w_q, w_k, w_v, w_o):
    x = np.asarray(x); position_ids = np.asarray(position_ids)
    w_q = np.asarray(w_q); w_k = np.asarray(w_k)
    w_v = np.asarray(w_v); w_o = np.asarray(w_o)

    if "nc" not in _cache:
        _cache["nc"] = _build()
    nc = _cache["nc"]

    in_maps = _prep_shards(x, position_ids, w_q, w_k, w_v, w_o)
    res = run_bass_kernel_spmd(nc, in_maps, core_ids=list(range(NC_CORES)))

    outT = np.zeros((NSB, NDT, 128, 512), np.float64)
    ks, vs = [], []
    for i in range(NC_CORES):
        r = res.results[i]
        outT += r["outT"].astype(np.float64)
        ks.append(np.ascontiguousarray(r["kT"].T))                 # [S, 128]
        vs.append(r["vout"].astype(np.float32).reshape(S, HD))
    out = outT.transpose(1, 2, 0, 3).reshape(D_MODEL, S).T.astype(np.float32)
    out = out.reshape(1, S, D_MODEL)
    k = np.stack(ks)[None].astype(np.float32)                      # [1,8,S,128]
    v = np.stack(vs)[None]
    return out, k, v


# revision 50
# speedup vs baseline: 1.0517x; 1.0011x over previous
"""GQA attention (32 Q heads / 8 KV heads, head_dim 128, d_model 2560, s=2048)
with RoPE, tensor-parallel across 8 TRN2 NeuronCores.

Sharding: core i owns Q heads 4i..4i+3 and KV head i (w_q/w_k/w_v sharded on
the head output dim, w_o on its input dim). Each core produces a full-shape
partial of the output projection; the partials are summed on the host (the
"all-reduce after w_o" of the hint, done at unshard time), so no on-device
collective is needed and outputs (k/v caches, out rows) are disjoint/partial.

Precision: the softmax logits here have std ~2.5e3 (softmax is near one-hot),
so Q/K projections and QK^T are computed as 3-pass split-bf16 matmuls
(a1b1+a1b2+a2b1, fp32-equivalent to ~2^-18) while the V chain (V proj,
attn@V, O proj) runs in plain bf16. Measured end-to-end rel err ~3e-3.

Layout: everything runs transposed ([dim, seq] on chip) so that projections,
scores^T, attn@V and the O projection all chain without any transposes.
Softmax runs over the partition axis via gpsimd C-axis reduce + broadcast.
"""
import sys
sys.path.insert(0, '/opt/trn_rl_repo')

import numpy as np
import ml_dtypes

import concourse.bass as bass
import concourse.tile as tile
from concourse import bacc, mybir
from concourse.bass_utils import run_bass_kernel_spmd

BF16 = ml_dtypes.bfloat16
F32 = mybir.dt.float32
BF = mybir.dt.bfloat16

D_MODEL = 2560
NUM_HEADS = 32
NUM_KV = 8
HD = 128
S = 2048
THETA = 5000000.0
NC_CORES = 8
NDT = D_MODEL // 128      # 20 contraction tiles
NSB = S // 512            # 4 sequence blocks of 512
NTT = S // 128            # 16 key tiles of 128
NM = 4                    # q heads per core
SCALE = 1.0 / np.sqrt(HD)
NEG = -1.0e30

_cache = {}


def _build():
    nc = bacc.Bacc("TRN2", target_bir_lowering=False, debug=False,
                   num_devices=NC_CORES)
    dp = nc.declare_dram_parameter
    x1_e = dp("x1", [NSB, NDT, 128, 512], BF, isOutput=False)
    x2_e = dp("x2", [NSB, NDT, 128, 512], BF, isOutput=False)
    wq1_e = dp("wq1", [NM, 128, NDT * 128], BF, isOutput=False)
    wq2_e = dp("wq2", [NM, 128, NDT * 128], BF, isOutput=False)
    wk1_e = dp("wk1", [128, NDT * 128], BF, isOutput=False)
    wk2_e = dp("wk2", [128, NDT * 128], BF, isOutput=False)
    wv_e = dp("wv", [128, NDT * 128], BF, isOutput=False)
    wo_e = dp("wo", [128, NM * NDT * 128], BF, isOutput=False)
    cos_e = dp("cosT", [128, S], F32, isOutput=False)  # rows 0:64 cos, 64:128 sin
    msk_e = dp("masks", [128, 4 * 512], BF, isOutput=False)
    out_e = dp("outT", [NSB, NDT, 128, 512], F32, isOutput=True)
    kt_e = dp("kT", [128, S], F32, isOutput=True)
    v_e = dp("vout", [NTT, 128, 128], BF, isOutput=True)

    with tile.TileContext(nc) as tc:
        _body(nc, tc, x1_e, x2_e, wq1_e, wq2_e, wk1_e, wk2_e, wv_e, wo_e,
              cos_e, msk_e, out_e, kt_e, v_e)
    nc.compile()
    return nc


def _body(nc, tc, x1_e, x2_e, wq1_e, wq2_e, wk1_e, wk2_e, wv_e, wo_e,
          cos_e, msk_e, out_e, kt_e, v_e):
    AF = mybir.ActivationFunctionType
    OP = mybir.AluOpType
    from concourse import bass_isa
    from concourse.masks import make_identity
    RMAX = bass_isa.ReduceOp.max
    RADD = bass_isa.ReduceOp.add
    with (
        tc.tile_pool(name="wres", bufs=1) as wres,      # resident weights/tables
        tc.tile_pool(name="kv", bufs=1) as kvp,         # k1,k2,v resident
        tc.tile_pool(name="xp", bufs=NDT) as xp,        # x1 resident / x2 stream
        tc.tile_pool(name="stg", bufs=1) as stg,      # fp32 score staging
        tc.tile_pool(name="sml", bufs=2) as sml,        # small rotating scratch
        tc.tile_pool(name="att", bufs=2) as att,        # bf16 attn weights
        tc.tile_pool(name="sts", bufs=1) as sts,        # softmax stats
        tc.tile_pool(name="ao", bufs=4) as aop,         # attn out per head
        tc.tile_pool(name="ost", bufs=2) as ostp,       # out staging
        tc.tile_pool(name="pacc", bufs=3, space="PSUM") as pacc,
        tc.tile_pool(name="psc", bufs=2, space="PSUM") as pscp,
        tc.tile_pool(name="pav", bufs=1, space="PSUM") as pavp,
        tc.tile_pool(name="po", bufs=2, space="PSUM") as pop,
    ):
        # resident loads (partition-major dram layouts; all contiguous DMAs)
        wk1 = wres.tile([128, NDT, 128], BF)
        wk2 = wres.tile([128, NDT, 128], BF)
        wv = wres.tile([128, NDT, 128], BF)
        nc.scalar.dma_start(wk1[:], wk1_e[:].rearrange("p (a b) -> p a b", b=128))
        nc.scalar.dma_start(wk2[:], wk2_e[:].rearrange("p (a b) -> p a b", b=128))
        nc.scalar.dma_start(wv[:], wv_e[:].rearrange("p (a b) -> p a b", b=128))
        wq1m, wq2m = [], []
        for m in range(NM):
            w1 = wres.tile([128, NDT, 128], BF, name=f"wq1m{m}")
            w2 = wres.tile([128, NDT, 128], BF, name=f"wq2m{m}")
            nc.scalar.dma_start(w1[:], wq1_e[m].rearrange("p (a b) -> p a b", b=128))
            nc.scalar.dma_start(w2[:], wq2_e[m].rearrange("p (a b) -> p a b", b=128))
            wq1m.append(w1)
            wq2m.append(w2)
        tbl = wres.tile([128, S], F32)   # rows 0:64 cos, rows 64:128 sin
        masks = wres.tile([128, 4, 512], BF)
        ident = wres.tile([128, 128], BF)
        make_identity(nc, ident)
        nc.scalar.dma_start(tbl[:], cos_e[:])
        nc.scalar.dma_start(masks[:], msk_e[:].rearrange("p (a b) -> p a b", b=512))
        wo = wres.tile([128, NM, NDT, 128], BF)
        nc.scalar.dma_start(wo[:], wo_e[:].rearrange("p (a b c) -> p a b c",
                                                   b=NDT, c=128))

        stgA = stg.tile([128, 8 * 512], F32)   # score staging, key tiles 0..7
        stgB = stg.tile([128, 8 * 512], F32)   # score staging, key tiles 8..15
        k1 = kvp.tile([128, S], BF)
        k2 = kvp.tile([128, S], BF)
        vsb = kvp.tile([128, S], BF)   # [t_local, tt*128+e]

        def rope(psrc, cs):
            """rope a [128,512] psum tile -> fp32 sbuf tile (5 DVE ops)"""
            t1 = sml.tile([128, 512], F32, tag="t1", bufs=1)
            t2 = sml.tile([128, 512], F32, tag="t2", bufs=1)
            r = sml.tile([128, 512], F32, tag="rope")
            nc.vector.tensor_mul(t1[0:64, :], psrc[0:64, :], tbl[0:64, cs])
            nc.vector.tensor_mul(t1[64:128, :], psrc[64:128, :], tbl[0:64, cs])
            nc.vector.tensor_mul(t2[0:64, :], psrc[64:128, :], tbl[64:128, cs])
            nc.vector.tensor_mul(t2[64:128, :], psrc[0:64, :], tbl[64:128, cs])
            nc.vector.tensor_sub(r[0:64, :], t1[0:64, :], t2[0:64, :])
            nc.vector.tensor_add(r[64:128, :], t1[64:128, :], t2[64:128, :])
            return r

        for sb in range(NSB):
            cs = bass.ts(sb, 512)       # column slice for this seq block
            nt = 4 * (sb + 1)           # causal key tiles

            x1t, x2t0 = [], []
            for dt in range(NDT):
                a = xp.tile([128, 512], BF, tag="x1")
                nc.sync.dma_start(a[:], x1_e[sb, dt])
                x1t.append(a)
                b = xp.tile([128, 512], BF, tag="x2", bufs=8, name=f"x2p{dt}")
                nc.sync.dma_start(b[:], x2_e[sb, dt])
                x2t0.append(b)

            def x2tile(dt):
                b = xp.tile([128, 512], BF, tag="x2", bufs=8)
                nc.sync.dma_start(b[:], x2_e[sb, dt])
                return b

            # ---- K (x1 passes) + vT in one dt pass; K x2 pass rides pair 0 ----
            pk = pacc.tile([128, 512], F32, tag="acc")
            pvT = pacc.tile([128, 512], F32, tag="acc")
            for dt in range(NDT):
                fst, lst = dt == 0, dt == NDT - 1
                nc.tensor.matmul(pk[:], wk1[:, dt], x1t[dt][:],
                                 start=fst, stop=False)
                nc.tensor.matmul(pk[:], wk2[:, dt], x1t[dt][:],
                                 start=False, stop=False)
                nc.tensor.matmul(pvT[:], wv[:, dt], x1t[dt][:],
                                 start=fst, stop=lst)

            vT_sb = sml.tile([128, 512], BF, tag="vts")
            nc.scalar.activation(vT_sb[:], pvT[:], AF.Copy)
            for tt in range(4):
                g = 4 * sb + tt
                tp = pscp.tile([128, 128], BF, tag="sc", name=f"tp{g}")
                nc.tensor.transpose(tp[:], vT_sb[:, bass.ts(tt, 128)], ident[:])
                nc.scalar.activation(vsb[:, bass.ts(g, 128)], tp[:], AF.Copy)
                nc.sync.dma_start(v_e[g], vsb[:, bass.ts(g, 128)])

            # ---- Q projection in head pairs + rope + scaled split ----
            q1h, q2h = [None] * NM, [None] * NM
            for pair in range(2):
                mA, mB = 2 * pair, 2 * pair + 1
                pqA = pacc.tile([128, 512], F32, tag="acc", name=f"pqA{pair}")
                pqB = pacc.tile([128, 512], F32, tag="acc", name=f"pqB{pair}")
                for dt in range(NDT):
                    fst = dt == 0
                    x2 = x2t0[dt] if pair == 0 else x2tile(dt)
                    if pair == 0:
                        nc.tensor.matmul(pk[:], wk1[:, dt], x2[:],
                                         start=False, stop=(dt == NDT - 1))
                    nc.tensor.matmul(pqA[:], wq1m[mA][:, dt], x2[:],
                                     start=fst, stop=False)
                    nc.tensor.matmul(pqB[:], wq1m[mB][:, dt], x2[:],
                                     start=fst, stop=False)
                if pair == 0:
                    kTs = rope(pk, cs)
                    nc.scalar.activation(k1[:, cs], kTs[:], AF.Copy)
                    k1f = sml.tile([128, 512], F32, tag="sub")
                    nc.scalar.activation(k1f[:], k1[:, cs], AF.Copy)
                    nc.vector.tensor_sub(k2[:, cs], kTs[:], k1f[:])
                    nc.sync.dma_start(kt_e[:, cs], kTs[:])
                for dt in range(NDT):
                    lst = dt == NDT - 1
                    nc.tensor.matmul(pqA[:], wq1m[mA][:, dt], x1t[dt][:],
                                     start=False, stop=False)
                    nc.tensor.matmul(pqA[:], wq2m[mA][:, dt], x1t[dt][:],
                                     start=False, stop=lst)
                for dt in range(NDT):
                    lst = dt == NDT - 1
                    nc.tensor.matmul(pqB[:], wq1m[mB][:, dt], x1t[dt][:],
                                     start=False, stop=False)
                    nc.tensor.matmul(pqB[:], wq2m[mB][:, dt], x1t[dt][:],
                                     start=False, stop=lst)
                for m, pq in ((mA, pqA), (mB, pqB)):
                    qTs = rope(pq, cs)
                    q1 = aop.tile([128, 512], BF, tag="q1")
                    q2 = aop.tile([128, 512], BF, tag="q2")
                    nc.scalar.activation(q1[:], qTs[:], AF.Copy, scale=SCALE)
                    q1f = sml.tile([128, 512], F32, tag="sub")
                    nc.scalar.activation(q1f[:], q1[:], AF.Copy)
                    nc.vector.scalar_tensor_tensor(q2[:], qTs[:], SCALE, q1f[:],
                                                   op0=OP.mult, op1=OP.subtract)
                    q1h[m], q2h[m] = q1, q2

            # ---- attention per head ----
            aout = []
            for h in range(NM):
                amax = sts.tile([128, 512], F32, tag="amax", bufs=2)
                for tt in range(nt):
                    ks = bass.ts(tt, 128)
                    ps = pscp.tile([128, 512], F32, tag="sc")
                    nc.tensor.matmul(ps[:], k1[:, ks], q1h[h][:],
                                     start=True, stop=False)
                    nc.tensor.matmul(ps[:], k1[:, ks], q2h[h][:],
                                     start=False, stop=False)
                    nc.tensor.matmul(ps[:], k2[:, ks], q1h[h][:],
                                     start=False, stop=True)
                    st = (stgA if tt < 8 else stgB)[:, bass.ts(tt % 8, 512)]
                    j = tt - 4 * sb
                    if j >= 0:
                        nc.vector.tensor_add(st, ps[:], masks[:, j])
                    else:
                        nc.scalar.activation(st, ps[:], AF.Copy)
                # strided reduce per quarter (4 tiles) so the chain tail is short
                na = min(nt, 8)
                nq = nt // 4
                amaxq = sts.tile([128, 512], F32, tag="amaxB")
                for q in range(nq):
                    half = stgA if q < 2 else stgB
                    off = (q % 2) * 4 * 512
                    sv = half[:, off:off + 4 * 512].rearrange(
                        "p (t c) -> p c t", c=512)
                    if q == 0:
                        nc.vector.tensor_reduce(
                            amax[:], sv, axis=mybir.AxisListType.X, op=OP.max)
                    else:
                        nc.vector.tensor_reduce(
                            amaxq[:], sv, axis=mybir.AxisListType.X, op=OP.max)
                        nc.vector.tensor_max(amax[:], amax[:], amaxq[:])
                mxb = sts.tile([128, 512], F32, tag="mxb", bufs=2)
                nc.gpsimd.partition_all_reduce(mxb[:], amax[:], channels=128,
                                               reduce_op=RMAX)
                pv2 = pavp.tile([128, 512], F32, tag="av")
                asum = sts.tile([128, 512], F32, tag="asum")
                # first tile subtracted alone so exp/attnV can start immediately
                nc.vector.tensor_sub(stgA[:, 0:512], stgA[:, 0:512], mxb[:])
                blkA = stgA[:, 512:na * 512].rearrange("p (t c) -> p t c", c=512)
                nc.vector.tensor_sub(
                    blkA, blkA,
                    mxb[:].unsqueeze(1).broadcast_to([128, na - 1, 512]))
                if nt > 8:
                    blkB = stgB[:, 0:(nt - 8) * 512].rearrange(
                        "p (t c) -> p t c", c=512)
                    nc.vector.tensor_sub(
                        blkB, blkB,
                        mxb[:].unsqueeze(1).broadcast_to([128, nt - 8, 512]))
                for tt in range(nt):
                    sl = (stgA if tt < 8 else stgB)[:, bass.ts(tt % 8, 512)]
                    at = att.tile([128, 512], BF, tag="at")
                    nc.scalar.activation(at[:], sl, AF.Exp)
                    if tt == 0:
                        nc.vector.tensor_copy(asum[:], at[:])
                    else:
                        nc.vector.tensor_add(asum[:], asum[:], at[:])
                    nc.tensor.matmul(pv2[:], vsb[:, bass.ts(tt, 128)], at[:],
                                     start=(tt == 0), stop=(tt == nt - 1))
                dsb = sts.tile([128, 512], F32, tag="dsb")
                rcb = sts.tile([128, 512], F32, tag="rcb")
                nc.gpsimd.partition_all_reduce(dsb[:], asum[:], channels=128,
                                               reduce_op=RADD)
                nc.vector.reciprocal(rcb[:], dsb[:])
                ah = aop.tile([128, 512], BF, tag="aout")
                nc.vector.tensor_mul(ah[:], pv2[:], rcb[:])
                aout.append(ah)

            # ---- O projection ----
            for dt in range(NDT):
                po = pop.tile([128, 512], F32, tag="o")
                for et in range(NM):
                    nc.tensor.matmul(po[:], wo[:, et, dt], aout[et][:],
                                     start=(et == 0), stop=(et == NM - 1))
                ost = ostp.tile([128, 512], F32, tag="ost")
                nc.scalar.activation(ost[:], po[:], AF.Copy)
                nc.sync.dma_start(out_e[sb, dt], ost[:])


def _prep_shards(x, position_ids, w_q, w_k, w_v, w_o):
    xs = np.ascontiguousarray(x.reshape(S, D_MODEL).T.astype(np.float32))
    x1 = xs.astype(BF16)
    x2 = (xs - x1.astype(np.float32)).astype(BF16)

    def xfmt(a):
        return np.ascontiguousarray(
            a.reshape(NDT, 128, NSB, 512).transpose(2, 0, 1, 3))

    x1s, x2s = xfmt(x1), xfmt(x2)

    pos = position_ids.astype(np.float32)
    inv = 1.0 / (THETA ** (np.arange(0, HD, 2, dtype=np.float32) / HD))  # 64
    ang = pos[None, :] * inv[:, None]          # [64, S]
    cosT = np.empty((128, S), np.float32)      # rows 0:64 cos, 64:128 sin
    cosT[0:64] = np.cos(ang)
    cosT[64:128] = np.sin(ang)

    masks = np.zeros((4, 128, 512), np.float32)  # cast to bf16 below
    jj = np.arange(128)
    cc = np.arange(512)
    for j in range(4):
        masks[j][(128 * j + jj)[:, None] > cc[None, :]] = NEG

    in_maps = []
    for i in range(NC_CORES):
        wq = np.ascontiguousarray(w_q[512 * i:512 * (i + 1)].T)   # [2560,512]
        wq1 = wq.astype(BF16)
        wq2 = (wq - wq1.astype(np.float32)).astype(BF16)

        def qfmt(a):
            # [2560, 512] -> [m, p, dt*128]: A[m, p, dt*128+c] = a[dt*128+p, m*128+c]
            return np.ascontiguousarray(
                a.reshape(NDT, 128, NM, 128).transpose(2, 1, 0, 3).reshape(
                    NM, 128, NDT * 128))

        def kfmt(a):
            # [2560, 128] -> [p, dt*128]: A[p, dt*128+c] = a[dt*128+p, c]
            return np.ascontiguousarray(
                a.reshape(NDT, 128, 128).transpose(1, 0, 2).reshape(128, NDT * 128))

        wk = np.ascontiguousarray(w_k[128 * i:128 * (i + 1)].T)   # [2560,128]
        wk1 = wk.astype(BF16)
        wk2 = (wk - wk1.astype(np.float32)).astype(BF16)
        wv = np.ascontiguousarray(
            w_v[128 * i:128 * (i + 1)].T).astype(BF16)
        wo = np.ascontiguousarray(w_o[:, 512 * i:512 * (i + 1)].T)  # [512,2560]
        # wo tile layout [p=e, (et, dt, c=d)]: A[e, et, dt, d] = wo[et*128+e, dt*128+d]
        wof = np.ascontiguousarray(
            wo.reshape(NM, 128, NDT, 128).transpose(1, 0, 2, 3).reshape(
                128, NM * NDT * 128)).astype(BF16)
        in_maps.append({
            "x1": x1s, "x2": x2s,
            "wq1": qfmt(wq1), "wq2": qfmt(wq2),
            "wk1": kfmt(wk1),
            "wk2": kfmt(wk2),
            "wv": kfmt(wv),
            "wo": wof,
            "cosT": cosT,
            "masks": masks.transpose(1, 0, 2).reshape(128, 4 * 512).astype(BF16),
        })
    return in_maps


def kernel(x, position_ids, w_q, w_k, w_v, w_o):
    x = np.asarray(x); position_ids = np.asarray(position_ids)
    w_q = np.asarray(w_q); w_k = np.asarray(w_k)
    w_v = np.asarray(w_v); w_o = np.asarray(w_o)

    if "nc" not in _cache:
        _cache["nc"] = _build()
    nc = _cache["nc"]

    in_maps = _prep_shards(x, position_ids, w_q, w_k, w_v, w_o)
    res = run_bass_kernel_spmd(nc, in_maps, core_ids=list(range(NC_CORES)))

    outT = np.zeros((NSB, NDT, 128, 512), np.float64)
    ks, vs = [], []
    for i in range(NC_CORES):
        r = res.results[i]
        outT += r["outT"].astype(np.float64)
        ks.append(np.ascontiguousarray(r["kT"].T))                 # [S, 128]
        vs.append(r["vout"].astype(np.float32).reshape(S, HD))
    out = outT.transpose(1, 2, 0, 3).reshape(D_MODEL, S).T.astype(np.float32)
    out = out.reshape(1, S, D_MODEL)
    k = np.stack(ks)[None].astype(np.float32)                      # [1,8,S,128]
    v = np.stack(vs)[None]
    return out, k, v
